# revision 1
# baseline (speedup 1.0000x reference)
"""Trainium2 Bass kernel for a causal pre-LN decoder block (B=2, T=2048, E=1024,
H=16, hd=64, dff=4096), SPMD over 8 NeuronCores.

Sharding: batch split across the two 4-core groups (cores 0-3 -> batch 0,
cores 4-7 -> batch 1). Within a group, attention is tensor-parallel over heads
(4 heads per core, full sequence), everything token-wise (LN, residuals, the
attention output projection and the whole FFN) is sequence-parallel (512 tokens
per core). Two small bf16 collectives glue the two shardings together:
an AllGather of h^T (each core's 512 normalized token columns) and an AllToAll
that redistributes per-head attention outputs o^T back to token owners.

The program is identical on every core; all per-core differences are carried by
the input data (token slice, head-sliced wq/wk/wv).

Matmul dtypes: residual-stream matmuls (FFN) run in float32r (full PE speed at
N>=512, ~16x more accurate than bf16); attention internals (QKV, scores, p@v,
w_proj) run in bf16, which only perturbs the small attn branch.
"""

import numpy as np
import ml_dtypes

import concourse.bacc as bacc
import concourse.mybir as mybir
import concourse.tile as tile
from concourse import bass_utils
from concourse.alu_op_type import AluOpType
from concourse.mybir import ActivationFunctionType as AFT
from bass_rust import AxisListType

B, T, E, H, HD, DFF = 2, 2048, 1024, 16, 64, 4096
NCORES, TP = 8, 4
TOWN = T // TP        # 512 tokens owned per core
NT = TOWN // 128      # 4 own token tiles
ET = E // 128         # 8 tiles along E
KT = T // 128         # 16 kv tiles over full T
QB = T // 512         # 4 query blocks of 512 over full T
HL = H // TP          # 4 local heads
FT = DFF // 128       # 32 tiles along dff
EPS = 1e-5

F32 = mybir.dt.float32
F32R = mybir.dt.float32r
BF16 = mybir.dt.bfloat16
RG = [[0, 1, 2, 3], [4, 5, 6, 7]]

_CACHE = {}


class _Stop(Exception):
    pass


def _layer_norm(nc, pool, out_slice, x_slice, g_rep, b_rep, tmp_tag):
    """out = (x - mean) / sqrt(var + EPS) * gamma + beta, rows = tokens."""
    st = pool.tile([128, 1], F32, tag=tmp_tag + "_s")
    nc.vector.reduce_sum(st[:], x_slice, AxisListType.X)
    nmean = pool.tile([128, 1], F32, tag=tmp_tag + "_m")
    nc.vector.tensor_scalar(nmean[:], st[:], -1.0 / E, None, op0=AluOpType.mult)
    xc = pool.tile([128, E], F32, tag=tmp_tag + "_xc")
    nc.vector.tensor_scalar(xc[:], x_slice, nmean[:], None, op0=AluOpType.add)
    sq = pool.tile([128, E], F32, tag=tmp_tag + "_sq", bufs=1)
    nc.vector.tensor_tensor(sq[:], xc[:], xc[:], op=AluOpType.mult)
    var = pool.tile([128, 1], F32, tag=tmp_tag + "_v")
    nc.vector.reduce_sum(var[:], sq[:], AxisListType.X)
    veps = pool.tile([128, 1], F32, tag=tmp_tag + "_ve")
    nc.vector.tensor_scalar(veps[:], var[:], 1.0 / E, EPS, op0=AluOpType.mult, op1=AluOpType.add)
    rv = pool.tile([128, 1], F32, tag=tmp_tag + "_rv")
    nc.vector.reciprocal(rv[:], veps[:])
    rstd = pool.tile([128, 1], F32, tag=tmp_tag + "_rs")
    nc.scalar.activation(rstd[:], rv[:], AFT.Sqrt)
    nc.vector.scalar_tensor_tensor(
        out_slice, xc[:], rstd[:], g_rep, op0=AluOpType.mult, op1=AluOpType.mult
    )
    nc.vector.tensor_tensor(out_slice, out_slice, b_rep, op=AluOpType.add)


def build(single=False, upto=99):
    ndev = 1 if single else NCORES
    nc = bacc.Bacc("TRN2", target_bir_lowering=False, debug=False, num_devices=ndev)

    def din(name, shape, dt):
        return nc.dram_tensor(name, shape, dt, kind="ExternalInput").ap()

    x_d = din("x_own", [TOWN, E], F32)
    wq_d = din("wq_s", [E, HL * HD], BF16)
    wk_d = din("wk_s", [E, HL * HD], BF16)
    wv_d = din("wv_s", [E, HL * HD], BF16)
    wp_d = din("w_proj", [2 * E, E], BF16)
    w1_d = din("w1", [DFF, E], F32R)  # host-reordered: row 128*ft+p, col (kt, m)
    w2_d = din("w2", [DFF, E], F32R)
    bp_d = din("b_proj", [1, E], F32)
    b1_d = din("b1", [DFF], F32)
    b2_d = din("b2", [1, E], F32)
    g1_d = din("gamma1", [1, E], F32)
    be1_d = din("beta1", [1, E], F32)
    g2_d = din("gamma2", [1, E], F32)
    be2_d = din("beta2", [1, E], F32)
    id_d = din("ident", [128, 128], F32)
    mk_d = din("mask_diag", [128, 4 * 512], BF16)
    out_d = nc.dram_tensor("out_own", [TOWN, E], F32, kind="ExternalOutput").ap()

    with tile.TileContext(nc) as tc:
        with (
            tc.tile_pool(name="dram", bufs=1, space="DRAM") as dram,
            tc.tile_pool(name="persist", bufs=1) as pp,
        ):
            def _emit():
                bounce1_in = dram.tile([E, TOWN], BF16)
                bounce1_out = dram.tile([TP * E, TOWN], BF16)
                bounce2_in = dram.tile([NCORES * 256, TOWN], BF16)
                bounce2_out = dram.tile([NCORES * 256, TOWN], BF16)

                ident = pp.tile([128, 128], F32)
                nc.sync.dma_start(ident[:], id_d[:])
                bp_rep = pp.tile([128, E], F32)
                nc.sync.dma_start(bp_rep[:], bp_d[0:1, :].to_broadcast([128, E]))
                wp_sb = pp.tile([128, 2 * ET, E], BF16)
                h_sb = pp.tile([128, NT, E], F32)
                h2_sb = pp.tile([128, NT, E], F32)

                # ---------------- P0/P1: load x, LN1 -> h ----------------
                w1s_cm = tc.tile_pool(name="w1s", bufs=4)
                w1p = w1s_cm.__enter__()
                w2s_cm = tc.tile_pool(name="w2s", bufs=2)
                w2p = w2s_cm.__enter__()
                attin_cm = tc.tile_pool(name="attin", bufs=1)
                ap_ = attin_cm.__enter__()
                pool_stack = [w1s_cm, w2s_cm, attin_cm]

                def _unwind():
                    while pool_stack:
                        pool_stack.pop().__exit__(None, None, None)

                def _run():
                    qT = ap_.tile([128, 2, T], BF16)      # q^T  [e', mt, t]
                    kT = ap_.tile([128, 2, T], BF16)
                    v_aug = ap_.tile([128, KT, HL, HD + 1], BF16)
                    oT = ap_.tile([128, 2, T], BF16)
                    with (
                        tc.tile_pool(name="src", bufs=1) as sp,
                        tc.tile_pool(name="lntmp", bufs=2) as lt,
                    ):
                        g1_rep = sp.tile([128, E], F32)
                        nc.sync.dma_start(g1_rep[:], g1_d[0:1, :].to_broadcast([128, E]))
                        be1_rep = sp.tile([128, E], F32)
                        nc.sync.dma_start(be1_rep[:], be1_d[0:1, :].to_broadcast([128, E]))

                        if upto >= 1:
                            for tt in range(NT):
                                xt = lt.tile([128, E], F32, tag="xt")
                                nc.sync.dma_start(xt[:], x_d[128 * tt : 128 * (tt + 1), :])
                                _layer_norm(
                                    nc, lt, h_sb[:, tt, :], xt[:], g1_rep[:], be1_rep[:], "ln1"
                                )

                        # ---------- P2: transpose h -> hT_own (bf16) ----------
                        hT_own = sp.tile([128, ET, TOWN], BF16)
                        with tc.tile_pool(name="pst", bufs=2, space="PSUM") as pst:
                            for tt in range(NT if upto >= 2 else 0):
                                for et in range(ET):
                                    ps = pst.tile([128, 128], F32, tag="tr")
                                    nc.tensor.transpose(
                                        ps[:], h_sb[:, tt, 128 * et : 128 * (et + 1)], ident[:]
                                    )
                                    nc.vector.tensor_copy(
                                        hT_own[:, et, 128 * tt : 128 * (tt + 1)], ps[:]
                                    )

                        # ---------- P3: AllGather h^T ----------
                        if upto < 3:
                            return
                        hT_full = sp.tile([128, ET, TP, TOWN], BF16)
                        nc.sync.dma_start(
                            bounce1_in.rearrange("(et p) t -> p et t", p=128), hT_own[:]
                        )
                        if single:
                            for r in range(TP):
                                nc.sync.dma_start(bounce1_out[E * r : E * (r + 1), :], bounce1_in[:])
                        else:
                            nc.gpsimd.collective_compute(
                                "AllGather", AluOpType.bypass, replica_groups=RG,
                                ins=[bounce1_in.opt()], outs=[bounce1_out.opt()],
                            )
                        for r in range(TP):
                            nc.sync.dma_start(
                                hT_full[:, :, r, :],
                                bounce1_out[E * r : E * (r + 1), :].rearrange(
                                    "(et p) t -> p et t", p=128
                                ),
                            )

                        # ---------- P4: QKV projections (bf16) ----------
                        wq_sb = sp.tile([128, ET, HL * HD], BF16)
                        nc.sync.dma_start(wq_sb[:], wq_d.rearrange("(kt p) m -> p kt m", p=128))
                        wk_sb = sp.tile([128, ET, HL * HD], BF16)
                        nc.sync.dma_start(wk_sb[:], wk_d.rearrange("(kt p) m -> p kt m", p=128))
                        wv_sb = sp.tile([128, ET, HL * HD], BF16)
                        nc.sync.dma_start(wv_sb[:], wv_d.rearrange("(kt p) m -> p kt m", p=128))
                        nc.sync.dma_start(wp_sb[:], wp_d.rearrange("(kt p) e -> p kt e", p=128))

                        if upto < 4:
                            return
                        with tc.tile_pool(name="pmm", bufs=4, space="PSUM") as pmm:
                            for dst, w_sb in ((qT, wq_sb), (kT, wk_sb)):
                                for mt in range(2):
                                    for r in range(TP):
                                        ps = pmm.tile([128, 512], F32, tag="qk")
                                        for kt in range(ET):
                                            nc.tensor.matmul(
                                                ps[:],
                                                w_sb[:, kt, 128 * mt : 128 * (mt + 1)],
                                                hT_full[:, kt, r, :],
                                                start=(kt == 0), stop=(kt == ET - 1),
                                            )
                                        nc.vector.tensor_copy(
                                            dst[:, mt, 512 * r : 512 * (r + 1)], ps[:]
                                        )
                            for t16 in range(KT):
                                r, m = t16 // NT, t16 % NT
                                ps = pmm.tile([128, 512], F32, tag="qk")
                                for kt in range(ET):
                                    nc.tensor.matmul(
                                        ps[:, 0 : HL * HD],
                                        hT_full[:, kt, r, 128 * m : 128 * (m + 1)],
                                        wv_sb[:, kt, :],
                                        start=(kt == 0), stop=(kt == ET - 1),
                                    )
                                nc.vector.tensor_copy(
                                    v_aug[:, t16, :, 0:HD],
                                    ps[:, 0 : HL * HD].rearrange("p (hh d) -> p hh d", d=HD),
                                )
                        nc.vector.memset(v_aug[:, :, :, HD], 1.0)

                    # ---------------- P5: attention ----------------
                    if upto < 5:
                        return

                    with (
                        tc.tile_pool(name="work", bufs=2) as wp,
                        tc.tile_pool(name="worksm", bufs=2) as wsm,
                        tc.tile_pool(name="ps_s", bufs=2, space="PSUM") as pss,
                        tc.tile_pool(name="ps_o", bufs=2, space="PSUM") as pso,
                    ):
                        mask = wp.tile([128, 4 * 512], BF16, bufs=1)
                        nc.sync.dma_start(mask[:], mk_d[:])
                        for hh in range(HL):
                            pb = 64 * (hh % 2)
                            mt = hh // 2
                            for qb in range(QB):
                                u_sb = wp.tile([128, KT, 512], BF16, tag="u")
                                nkv = 4 * qb + 4
                                jt0 = 0
                                while jt0 < nkv:
                                    gw = min(3, nkv - jt0)
                                    ps = pss.tile([128, 3, 512], F32, tag="s")
                                    for m in range(gw):
                                        jt = jt0 + m
                                        co = max(0, 128 * (jt - 4 * qb))  # causal col start
                                        nc.tensor.matmul(
                                            ps[:, m, co:512],
                                            kT[pb : pb + 64, mt, 128 * jt : 128 * (jt + 1)],
                                            qT[pb : pb + 64, mt, 512 * qb + co : 512 * (qb + 1)],
                                            start=True, stop=True,
                                        )
                                    if 128 * (jt0 + gw - 1 - 4 * qb) <= 0:
                                        nc.scalar.activation(
                                            u_sb[:, jt0 : jt0 + gw, :], ps[:, 0:gw, :],
                                            AFT.Exp, scale=1.0 / np.sqrt(HD),
                                        )
                                    else:
                                        for m in range(gw):
                                            jt = jt0 + m
                                            co = max(0, 128 * (jt - 4 * qb))
                                            nc.scalar.activation(
                                                u_sb[:, jt, co:512], ps[:, m, co:512],
                                                AFT.Exp, scale=1.0 / np.sqrt(HD),
                                            )
                                            if co > 0:
                                                nc.vector.memset(u_sb[:, jt, 0:co], 0.0)
                                    jt0 += gw
                                # causal fixup: one fused multiplicative mask over the
                                # diagonal quad (zeroes below-left, triangular on diag)
                                nc.vector.tensor_tensor(
                                    u_sb[:, 4 * qb : 4 * qb + 4, :],
                                    u_sb[:, 4 * qb : 4 * qb + 4, :],
                                    mask[:], op=AluOpType.mult,
                                )
                                po = pso.tile([128, 512], F32, tag="o")
                                for jt in range(nkv):
                                    nc.tensor.matmul(
                                        po[0 : HD + 1, :],
                                        v_aug[:, jt, hh, :],
                                        u_sb[:, jt, :],
                                        start=(jt == 0), stop=(jt == nkv - 1),
                                    )
                                rz = wsm.tile([1, 512], F32, tag="rz")
                                nc.vector.reciprocal(rz[:], po[HD : HD + 1, :])
                                rz_rep = wsm.tile([64, 512], F32, tag="rzr")
                                nc.gpsimd.partition_broadcast(rz_rep[:], rz[:])
                                nc.vector.tensor_tensor(
                                    oT[pb : pb + 64, mt, 512 * qb : 512 * (qb + 1)],
                                    po[0:HD, :], rz_rep[:], op=AluOpType.mult,
                                )

                    # ---------- P6: AllToAll o^T (heads -> token owners) ----------
                    if upto < 6:
                        return
                    for dst in range(NCORES):
                        d = dst % TP
                        nc.sync.dma_start(
                            bounce2_in[256 * dst : 256 * (dst + 1), :].rearrange(
                                "(mt p) t -> p mt t", p=128
                            ),
                            oT[:, :, 512 * d : 512 * (d + 1)],
                        )
                    pool_stack.pop().__exit__(None, None, None)  # attin
                    if single:
                        nc.sync.dma_start(bounce2_out[:], bounce2_in[:])
                    else:
                        nc.gpsimd.collective_compute(
                            "AllToAll", AluOpType.bypass, replica_groups=[list(range(NCORES))],
                            ins=[bounce2_in.opt()], outs=[bounce2_out.opt()],
                        )

                    # ---------- P7: w_proj + residual, P8: LN2 ----------
                    if upto < 7:
                        return
                    with (
                        tc.tile_pool(name="proj", bufs=1) as pj,
                        tc.tile_pool(name="lntmp2", bufs=2) as lt2,
                    ):
                        oT_own = pj.tile([128, 2 * ET, TOWN], BF16)
                        for tt in range(NT):
                            nc.sync.dma_start(
                                oT_own[:, :, 128 * tt : 128 * (tt + 1)],
                                bounce2_out[:, 128 * tt : 128 * (tt + 1)].rearrange(
                                    "(et p) t -> p et t", p=128
                                ),
                            )
                        x2_sb = pj.tile([128, NT, E], F32)
                        with tc.tile_pool(name="pmm2", bufs=4, space="PSUM") as pmm2:
                            for tt in range(NT):
                                for nh in range(2):
                                    ps = pmm2.tile([128, 512], F32, tag="ap")
                                    for kt in range(2 * ET):
                                        nc.tensor.matmul(
                                            ps[:],
                                            oT_own[:, kt, 128 * tt : 128 * (tt + 1)],
                                            wp_sb[:, kt, 512 * nh : 512 * (nh + 1)],
                                            start=(kt == 0), stop=(kt == 2 * ET - 1),
                                        )
                                    sl = slice(512 * nh, 512 * (nh + 1))
                                    nc.vector.tensor_tensor(
                                        x2_sb[:, tt, sl], ps[:], h_sb[:, tt, sl], op=AluOpType.add
                                    )
                                    nc.vector.tensor_tensor(
                                        x2_sb[:, tt, sl], x2_sb[:, tt, sl], bp_rep[:, sl],
                                        op=AluOpType.add,
                                    )
                        g2_rep = pj.tile([128, E], F32)
                        nc.sync.dma_start(g2_rep[:], g2_d[0:1, :].to_broadcast([128, E]))
                        be2_rep = pj.tile([128, E], F32)
                        nc.sync.dma_start(be2_rep[:], be2_d[0:1, :].to_broadcast([128, E]))
                        for tt in range(NT):
                            _layer_norm(
                                nc, lt2, h2_sb[:, tt, :], x2_sb[:, tt, :], g2_rep[:], be2_rep[:], "ln2"
                            )

                    # ---------- P9-P11: FFN (float32r) ----------
                    if upto < 9:
                        return
                    with tc.tile_pool(name="ffn", bufs=1) as fp:
                        h2T = fp.tile([128, ET, TOWN], F32R)
                        with tc.tile_pool(name="pst2", bufs=2, space="PSUM") as pst2:
                            for tt in range(NT):
                                for et in range(ET):
                                    ps = pst2.tile([128, 128], F32, tag="tr2")
                                    nc.tensor.transpose(
                                        ps[:], h2_sb[:, tt, 128 * et : 128 * (et + 1)], ident[:]
                                    )
                                    nc.vector.tensor_copy(
                                        h2T[:, et, 128 * tt : 128 * (tt + 1)], ps[:]
                                    )
                        b1_sb = fp.tile([128, FT], F32)
                        nc.sync.dma_start(b1_sb[:], b1_d.rearrange("(ft p) -> p ft", p=128))
                        aT = fp.tile([128, FT, TOWN], F32R)
                        with tc.tile_pool(name="pf", bufs=3, space="PSUM") as pf:
                            for ft in range(FT if upto >= 10 else 0):
                                w1t = w1p.tile([128, ET, 128], F32R, tag="w1")
                                nc.sync.dma_start(
                                    w1t[:],
                                    w1_d[128 * ft : 128 * (ft + 1), :].rearrange(
                                        "p (kt m) -> p kt m", kt=ET
                                    ),
                                )
                                ps = pf.tile([128, 512], F32, tag="f")
                                for kt in range(ET):
                                    nc.tensor.matmul(
                                        ps[:], w1t[:, kt, :], h2T[:, kt, :],
                                        start=(kt == 0), stop=(kt == ET - 1),
                                    )
                                # relu(ps + b1) on DVE, rounded to f32r
                                nc.vector.tensor_scalar(
                                    aT[:, ft, :], ps[:], b1_sb[:, ft : ft + 1], 0.0,
                                    op0=AluOpType.add, op1=AluOpType.max,
                                )
                        out_sb = h_sb
                        b2_rep = fp.tile([128, E], F32)
                        nc.sync.dma_start(b2_rep[:], b2_d[0:1, :].to_broadcast([128, E]))
                        for tt in range(NT):
                            nc.vector.tensor_tensor(
                                h2_sb[:, tt, :], h2_sb[:, tt, :], b2_rep[:], op=AluOpType.add
                            )
                        if upto < 11:
                            return
                        with tc.tile_pool(name="pff", bufs=8, space="PSUM") as pff:
                            accs = [pff.tile([128, 512], F32, tag="acc", name=f"acc{i}") for i in range(8)]
                            for ktf in range(FT):
                                w2t = w2p.tile([128, E], F32R, tag="w2")
                                nc.sync.dma_start(w2t[:], w2_d[128 * ktf : 128 * (ktf + 1), :])
                                for tt in range(NT):
                                    for nh in range(2):
                                        nc.tensor.matmul(
                                            accs[2 * tt + nh][:],
                                            aT[:, ktf, 128 * tt : 128 * (tt + 1)],
                                            w2t[:, 512 * nh : 512 * (nh + 1)],
                                            start=(ktf == 0), stop=(ktf == FT - 1),
                                        )
                            for tt in range(NT):
                                for nh in range(2):
                                    sl = slice(512 * nh, 512 * (nh + 1))
                                    nc.vector.tensor_tensor(
                                        out_sb[:, tt, sl], accs[2 * tt + nh][:],
                                        h2_sb[:, tt, sl], op=AluOpType.add,
                                    )
                        nc.sync.dma_start(out_d.rearrange("(tt p) e -> p tt e", p=128), out_sb[:])

                _run()
                _unwind()


            _emit()
    nc.compile()
    return nc


def _in_maps(inputs):
    x = np.asarray(inputs["x"], np.float32)
    wq = np.asarray(inputs["wq"], np.float32)
    wk = np.asarray(inputs["wk"], np.float32)
    wv = np.asarray(inputs["wv"], np.float32)
    w_proj = np.asarray(inputs["w_proj"], np.float32)
    w1 = np.asarray(inputs["w1"], np.float32)
    w1 = np.ascontiguousarray(
        w1.reshape(ET, 128, FT, 128).transpose(2, 1, 0, 3).reshape(DFF, E)
    )
    w2 = np.ascontiguousarray(np.asarray(inputs["w2"], np.float32))
    bp = np.asarray(inputs["b_proj"], np.float32).reshape(1, E)
    b1 = np.ascontiguousarray(np.asarray(inputs["b1"], np.float32))
    b2 = np.asarray(inputs["b2"], np.float32).reshape(1, E)
    g1 = np.asarray(inputs["gamma1"], np.float32).reshape(1, E)
    be1 = np.asarray(inputs["beta1"], np.float32).reshape(1, E)
    g2 = np.asarray(inputs["gamma2"], np.float32).reshape(1, E)
    be2 = np.asarray(inputs["beta2"], np.float32).reshape(1, E)
    # per-batch zero-padded w_proj: row block s (of 8) = w_proj rows of local
    # rank s%4 if core s belongs to this batch group, else zeros
    wpe = []
    for b in range(B):
        m = np.zeros((2 * E, E), np.float32)
        for s in range(NCORES):
            if s // TP == b:
                lr = s % TP
                m[256 * s : 256 * (s + 1)] = w_proj[256 * lr : 256 * (lr + 1)]
        wpe.append(m.astype(ml_dtypes.bfloat16))
    ident = np.eye(128, dtype=np.float32)
    # mask_diag[p, 512*m + c] = 1 if p <= c - 128*m  (diagonal quad causal mask)
    md = np.zeros((128, 4, 512), np.float32)
    for m in range(4):
        for pp_ in range(128):
            cstart = 128 * m + pp_
            if cstart < 512:
                md[pp_, m, cstart:] = 1.0
    mask_diag = md.reshape(128, 2048).astype(ml_dtypes.bfloat16)

    maps = []
    for c in range(NCORES):
        b, j = c // TP, c % TP
        heads = slice(HL * j, HL * (j + 1))
        maps.append({
            "x_own": np.ascontiguousarray(x[b, TOWN * j : TOWN * (j + 1)]),
            "wq_s": np.ascontiguousarray(wq[heads].transpose(1, 0, 2).reshape(E, HL * HD)).astype(ml_dtypes.bfloat16),
            "wk_s": np.ascontiguousarray(wk[heads].transpose(1, 0, 2).reshape(E, HL * HD)).astype(ml_dtypes.bfloat16),
            "wv_s": np.ascontiguousarray(wv[heads].transpose(1, 0, 2).reshape(E, HL * HD)).astype(ml_dtypes.bfloat16),
            "w_proj": wpe[b], "w1": w1, "w2": w2,
            "b_proj": bp, "b1": b1, "b2": b2,
            "gamma1": g1, "beta1": be1, "gamma2": g2, "beta2": be2,
            "ident": ident, "mask_diag": mask_diag,
        })
    return maps


def kernel(**inputs) -> np.ndarray:
    if "nc" not in _CACHE:
        _CACHE["nc"] = build()
    nc = _CACHE["nc"]
    res = bass_utils.run_bass_kernel_spmd(
        nc, _in_maps(inputs), core_ids=list(range(NCORES))
    )
    out = np.empty((B, T, E), np.float32)
    for c in range(NCORES):
        b, j = c // TP, c % TP
        out[b, TOWN * j : TOWN * (j + 1)] = res.results[c]["out_own"]
    return out



# revision 12
# speedup vs baseline: 1.0874x; 1.0874x over previous
"""Trainium2 Bass kernel for a causal pre-LN decoder block (B=2, T=2048, E=1024,
H=16, hd=64, dff=4096), SPMD over 8 NeuronCores.

Sharding: batch split across the two 4-core groups (cores 0-3 -> batch 0,
cores 4-7 -> batch 1). Within a group, attention is tensor-parallel over heads
(4 heads per core, full sequence), everything token-wise (LN, residuals, the
attention output projection and the whole FFN) is sequence-parallel (512 tokens
per core). Two small bf16 collectives glue the two shardings together:
an AllGather of h^T (each core's 512 normalized token columns) and an AllToAll
that redistributes per-head attention outputs o^T back to token owners.

The program is identical on every core; all per-core differences are carried by
the input data (token slice, head-sliced wq/wk/wv).

Matmul dtypes: residual-stream matmuls (FFN) run in float32r (full PE speed at
N>=512, ~16x more accurate than bf16); attention internals (QKV, scores, p@v,
w_proj) run in bf16, which only perturbs the small attn branch.
"""

import numpy as np
import ml_dtypes

import concourse.bacc as bacc
import concourse.mybir as mybir
import concourse.tile as tile
from concourse import bass_utils
from concourse.alu_op_type import AluOpType
from concourse.mybir import ActivationFunctionType as AFT
from bass_rust import AxisListType

B, T, E, H, HD, DFF = 2, 2048, 1024, 16, 64, 4096
NCORES, TP = 8, 4
TOWN = T // TP        # 512 tokens owned per core
NT = TOWN // 128      # 4 own token tiles
ET = E // 128         # 8 tiles along E
KT = T // 128         # 16 kv tiles over full T
QB = T // 512         # 4 query blocks of 512 over full T
HL = H // TP          # 4 local heads
FT = DFF // 128       # 32 tiles along dff
EPS = 1e-5

F32 = mybir.dt.float32
F32R = mybir.dt.float32r
BF16 = mybir.dt.bfloat16
RG = [[0, 1, 2, 3], [4, 5, 6, 7]]

_CACHE = {}


class _Stop(Exception):
    pass


def _layer_norm(nc, pool, out_slice, x_slice, g_rep, b_rep, tmp_tag):
    """out = (x - mean) / sqrt(var + EPS) * gamma + beta, rows = tokens."""
    st = pool.tile([128, 1], F32, tag=tmp_tag + "_s")
    nc.vector.reduce_sum(st[:], x_slice, AxisListType.X)
    nmean = pool.tile([128, 1], F32, tag=tmp_tag + "_m")
    nc.vector.tensor_scalar(nmean[:], st[:], -1.0 / E, None, op0=AluOpType.mult)
    xc = pool.tile([128, E], F32, tag=tmp_tag + "_xc")
    nc.vector.tensor_scalar(xc[:], x_slice, nmean[:], None, op0=AluOpType.add)
    sq = pool.tile([128, E], F32, tag=tmp_tag + "_sq", bufs=1)
    nc.vector.tensor_tensor(sq[:], xc[:], xc[:], op=AluOpType.mult)
    var = pool.tile([128, 1], F32, tag=tmp_tag + "_v")
    nc.vector.reduce_sum(var[:], sq[:], AxisListType.X)
    veps = pool.tile([128, 1], F32, tag=tmp_tag + "_ve")
    nc.vector.tensor_scalar(veps[:], var[:], 1.0 / E, EPS, op0=AluOpType.mult, op1=AluOpType.add)
    rv = pool.tile([128, 1], F32, tag=tmp_tag + "_rv")
    nc.vector.reciprocal(rv[:], veps[:])
    rstd = pool.tile([128, 1], F32, tag=tmp_tag + "_rs")
    nc.scalar.activation(rstd[:], rv[:], AFT.Sqrt)
    nc.vector.scalar_tensor_tensor(
        out_slice, xc[:], rstd[:], g_rep, op0=AluOpType.mult, op1=AluOpType.mult
    )
    nc.vector.tensor_tensor(out_slice, out_slice, b_rep, op=AluOpType.add)


def build(single=False, upto=99):
    ndev = 1 if single else NCORES
    nc = bacc.Bacc("TRN2", target_bir_lowering=False, debug=False, num_devices=ndev)

    def din(name, shape, dt):
        return nc.dram_tensor(name, shape, dt, kind="ExternalInput").ap()

    x_d = din("x_own", [TOWN, E], F32)
    wq_d = din("wq_s", [E, HL * HD], BF16)
    wk_d = din("wk_s", [E, HL * HD], BF16)
    wv_d = din("wv_s", [E, HL * HD], BF16)
    wp_d = din("w_proj", [2 * E, E], BF16)
    w1_d = din("w1", [DFF, E], BF16)  # host-reordered: row 128*ft+p, col (kt, m)
    w2_d = din("w2", [DFF, E], BF16)
    bp_d = din("b_proj", [1, E], F32)
    b1_d = din("b1", [DFF], F32)
    b2_d = din("b2", [1, E], F32)
    g1_d = din("gamma1", [1, E], F32)
    be1_d = din("beta1", [1, E], F32)
    g2_d = din("gamma2", [1, E], F32)
    be2_d = din("beta2", [1, E], F32)
    id_d = din("ident", [128, 128], F32)
    mk_d = din("mask_diag", [128, 4 * 512], BF16)
    out_d = nc.dram_tensor("out_own", [TOWN, E], F32, kind="ExternalOutput").ap()

    with tile.TileContext(nc) as tc:
        with (
            tc.tile_pool(name="dram", bufs=1, space="DRAM") as dram,
            tc.tile_pool(name="persist", bufs=1) as pp,
        ):
            def _emit():
                bounce1_in = dram.tile([E, TOWN], BF16)
                bounce1_out = dram.tile([TP * E, TOWN], BF16)
                bounce2_in = dram.tile([NCORES * 256, TOWN], BF16)
                bounce2_out = dram.tile([NCORES * 256, TOWN], BF16)

                ident = pp.tile([128, 128], F32)
                nc.sync.dma_start(ident[:], id_d[:])
                bp_rep = pp.tile([128, E], F32)
                nc.sync.dma_start(bp_rep[:], bp_d[0:1, :].to_broadcast([128, E]))
                wp_sb = pp.tile([128, 2 * ET, E], BF16)
                h_sb = pp.tile([128, NT, E], F32)
                h2_sb = pp.tile([128, NT, E], F32)

                # ---------------- P0/P1: load x, LN1 -> h ----------------
                w1s_cm = tc.tile_pool(name="w1s", bufs=4)
                w1p = w1s_cm.__enter__()
                w2s_cm = tc.tile_pool(name="w2s", bufs=6)
                w2p = w2s_cm.__enter__()
                attin_cm = tc.tile_pool(name="attin", bufs=1)
                ap_ = attin_cm.__enter__()
                pool_stack = [w1s_cm, w2s_cm, attin_cm]

                def _unwind():
                    while pool_stack:
                        pool_stack.pop().__exit__(None, None, None)

                def _run():
                    qT = ap_.tile([128, 2, T], BF16)      # q^T  [e', mt, t]
                    kT = ap_.tile([128, 2, T], BF16)
                    v_aug = ap_.tile([128, KT, HL, HD + 1], BF16)
                    oT = ap_.tile([128, 2, T], BF16)
                    with (
                        tc.tile_pool(name="src", bufs=1) as sp,
                        tc.tile_pool(name="lntmp", bufs=2) as lt,
                    ):
                        g1_rep = sp.tile([128, E], F32)
                        nc.sync.dma_start(g1_rep[:], g1_d[0:1, :].to_broadcast([128, E]))
                        be1_rep = sp.tile([128, E], F32)
                        nc.sync.dma_start(be1_rep[:], be1_d[0:1, :].to_broadcast([128, E]))

                        if upto >= 1:
                            for tt in range(NT):
                                xt = lt.tile([128, E], F32, tag="xt")
                                nc.sync.dma_start(xt[:], x_d[128 * tt : 128 * (tt + 1), :])
                                _layer_norm(
                                    nc, lt, h_sb[:, tt, :], xt[:], g1_rep[:], be1_rep[:], "ln1"
                                )

                        # ---------- P2: transpose h -> hT_own (bf16) ----------
                        hT_own = sp.tile([128, ET, TOWN], BF16)
                        with tc.tile_pool(name="pst", bufs=2, space="PSUM") as pst:
                            for tt in range(NT if upto >= 2 else 0):
                                for et in range(ET):
                                    ps = pst.tile([128, 128], F32, tag="tr")
                                    nc.tensor.transpose(
                                        ps[:], h_sb[:, tt, 128 * et : 128 * (et + 1)], ident[:]
                                    )
                                    nc.vector.tensor_copy(
                                        hT_own[:, et, 128 * tt : 128 * (tt + 1)], ps[:]
                                    )

                        # ---------- P3: AllGather h^T ----------
                        if upto < 3:
                            return
                        hT_full = sp.tile([128, ET, TP, TOWN], BF16)
                        nc.sync.dma_start(
                            bounce1_in.rearrange("(et p) t -> p et t", p=128), hT_own[:]
                        )
                        if single:
                            for r in range(TP):
                                nc.sync.dma_start(bounce1_out[E * r : E * (r + 1), :], bounce1_in[:])
                        else:
                            nc.gpsimd.collective_compute(
                                "AllGather", AluOpType.bypass, replica_groups=RG,
                                ins=[bounce1_in.opt()], outs=[bounce1_out.opt()],
                            )
                        for r in range(TP):
                            nc.sync.dma_start(
                                hT_full[:, :, r, :],
                                bounce1_out[E * r : E * (r + 1), :].rearrange(
                                    "(et p) t -> p et t", p=128
                                ),
                            )

                        # ---------- P4: QKV projections (bf16) ----------
                        wq_sb = sp.tile([128, ET, HL * HD], BF16)
                        nc.sync.dma_start(wq_sb[:], wq_d.rearrange("(kt p) m -> p kt m", p=128))
                        wk_sb = sp.tile([128, ET, HL * HD], BF16)
                        nc.sync.dma_start(wk_sb[:], wk_d.rearrange("(kt p) m -> p kt m", p=128))
                        wv_sb = sp.tile([128, ET, HL * HD], BF16)
                        nc.sync.dma_start(wv_sb[:], wv_d.rearrange("(kt p) m -> p kt m", p=128))
                        nc.sync.dma_start(wp_sb[:], wp_d.rearrange("(kt p) e -> p kt e", p=128))

                        if upto < 4:
                            return
                        with tc.tile_pool(name="pmm", bufs=4, space="PSUM") as pmm:
                            for dst, w_sb in ((qT, wq_sb), (kT, wk_sb)):
                                for mt in range(2):
                                    for r in range(TP):
                                        ps = pmm.tile([128, 512], F32, tag="qk")
                                        for kt in range(ET):
                                            nc.tensor.matmul(
                                                ps[:],
                                                w_sb[:, kt, 128 * mt : 128 * (mt + 1)],
                                                hT_full[:, kt, r, :],
                                                start=(kt == 0), stop=(kt == ET - 1),
                                            )
                                        nc.vector.tensor_copy(
                                            dst[:, mt, 512 * r : 512 * (r + 1)], ps[:]
                                        )
                            for t16 in range(KT):
                                r, m = t16 // NT, t16 % NT
                                ps = pmm.tile([128, 512], F32, tag="qk")
                                for kt in range(ET):
                                    nc.tensor.matmul(
                                        ps[:, 0 : HL * HD],
                                        hT_full[:, kt, r, 128 * m : 128 * (m + 1)],
                                        wv_sb[:, kt, :],
                                        start=(kt == 0), stop=(kt == ET - 1),
                                    )
                                nc.vector.tensor_copy(
                                    v_aug[:, t16, :, 0:HD],
                                    ps[:, 0 : HL * HD].rearrange("p (hh d) -> p hh d", d=HD),
                                )
                        nc.vector.memset(v_aug[:, :, :, HD], 1.0)

                    # ---------------- P5: attention ----------------
                    if upto < 5:
                        return

                    with (
                        tc.tile_pool(name="work", bufs=2) as wp,
                        tc.tile_pool(name="worksm", bufs=2) as wsm,
                        tc.tile_pool(name="ps_s", bufs=2, space="PSUM") as pss,
                        tc.tile_pool(name="ps_o", bufs=2, space="PSUM") as pso,
                    ):
                        mask = wp.tile([128, 4 * 512], BF16, bufs=1)
                        nc.sync.dma_start(mask[:], mk_d[:])
                        for hh in range(HL):
                            pb = 64 * (hh % 2)
                            mt = hh // 2
                            for qb in range(QB):
                                u_sb = wp.tile([128, KT, 512], BF16, tag="u")
                                nkv = 4 * qb + 4
                                jt0 = 0
                                while jt0 < nkv:
                                    gw = min(3, nkv - jt0)
                                    ps = pss.tile([128, 3, 512], F32, tag="s")
                                    for m in range(gw):
                                        jt = jt0 + m
                                        co = max(0, 128 * (jt - 4 * qb))  # causal col start
                                        nc.tensor.matmul(
                                            ps[:, m, co:512],
                                            kT[pb : pb + 64, mt, 128 * jt : 128 * (jt + 1)],
                                            qT[pb : pb + 64, mt, 512 * qb + co : 512 * (qb + 1)],
                                            start=True, stop=True,
                                        )
                                    if 128 * (jt0 + gw - 1 - 4 * qb) <= 0:
                                        nc.scalar.activation(
                                            u_sb[:, jt0 : jt0 + gw, :], ps[:, 0:gw, :],
                                            AFT.Exp, scale=1.0 / np.sqrt(HD),
                                        )
                                    else:
                                        for m in range(gw):
                                            jt = jt0 + m
                                            co = max(0, 128 * (jt - 4 * qb))
                                            nc.scalar.activation(
                                                u_sb[:, jt, co:512], ps[:, m, co:512],
                                                AFT.Exp, scale=1.0 / np.sqrt(HD),
                                            )
                                            if co > 0:
                                                nc.vector.memset(u_sb[:, jt, 0:co], 0.0)
                                    jt0 += gw
                                # causal fixup: one fused multiplicative mask over the
                                # diagonal quad (zeroes below-left, triangular on diag)
                                nc.vector.tensor_tensor(
                                    u_sb[:, 4 * qb : 4 * qb + 4, :],
                                    u_sb[:, 4 * qb : 4 * qb + 4, :],
                                    mask[:], op=AluOpType.mult,
                                )
                                po = pso.tile([128, 512], F32, tag="o")
                                for jt in range(nkv):
                                    nc.tensor.matmul(
                                        po[0 : HD + 1, :],
                                        v_aug[:, jt, hh, :],
                                        u_sb[:, jt, :],
                                        start=(jt == 0), stop=(jt == nkv - 1),
                                    )
                                rz = wsm.tile([1, 512], F32, tag="rz")
                                nc.vector.reciprocal(rz[:], po[HD : HD + 1, :])
                                rz_rep = wsm.tile([64, 512], F32, tag="rzr")
                                nc.gpsimd.partition_broadcast(rz_rep[:], rz[:])
                                nc.vector.tensor_tensor(
                                    oT[pb : pb + 64, mt, 512 * qb : 512 * (qb + 1)],
                                    po[0:HD, :], rz_rep[:], op=AluOpType.mult,
                                )

                    # ---------- P6: AllToAll o^T (heads -> token owners) ----------
                    if upto < 6:
                        return
                    for dst in range(NCORES):
                        d = dst % TP
                        nc.sync.dma_start(
                            bounce2_in[256 * dst : 256 * (dst + 1), :].rearrange(
                                "(mt p) t -> p mt t", p=128
                            ),
                            oT[:, :, 512 * d : 512 * (d + 1)],
                        )
                    pool_stack.pop().__exit__(None, None, None)  # attin
                    if single:
                        nc.sync.dma_start(bounce2_out[:], bounce2_in[:])
                    else:
                        nc.gpsimd.collective_compute(
                            "AllToAll", AluOpType.bypass, replica_groups=[list(range(NCORES))],
                            ins=[bounce2_in.opt()], outs=[bounce2_out.opt()],
                        )

                    # ---------- P7: w_proj + residual, P8: LN2 ----------
                    if upto < 7:
                        return
                    with (
                        tc.tile_pool(name="proj", bufs=1) as pj,
                        tc.tile_pool(name="lntmp2", bufs=2) as lt2,
                    ):
                        oT_own = pj.tile([128, 2 * ET, TOWN], BF16)
                        for tt in range(NT):
                            nc.sync.dma_start(
                                oT_own[:, :, 128 * tt : 128 * (tt + 1)],
                                bounce2_out[:, 128 * tt : 128 * (tt + 1)].rearrange(
                                    "(et p) t -> p et t", p=128
                                ),
                            )
                        x2_sb = pj.tile([128, NT, E], F32)
                        with tc.tile_pool(name="pmm2", bufs=4, space="PSUM") as pmm2:
                            for tt in range(NT):
                                for nh in range(2):
                                    ps = pmm2.tile([128, 512], F32, tag="ap")
                                    for kt in range(2 * ET):
                                        nc.tensor.matmul(
                                            ps[:],
                                            oT_own[:, kt, 128 * tt : 128 * (tt + 1)],
                                            wp_sb[:, kt, 512 * nh : 512 * (nh + 1)],
                                            start=(kt == 0), stop=(kt == 2 * ET - 1),
                                        )
                                    sl = slice(512 * nh, 512 * (nh + 1))
                                    nc.vector.tensor_tensor(
                                        x2_sb[:, tt, sl], ps[:], h_sb[:, tt, sl], op=AluOpType.add
                                    )
                                    nc.vector.tensor_tensor(
                                        x2_sb[:, tt, sl], x2_sb[:, tt, sl], bp_rep[:, sl],
                                        op=AluOpType.add,
                                    )
                        g2_rep = pj.tile([128, E], F32)
                        nc.sync.dma_start(g2_rep[:], g2_d[0:1, :].to_broadcast([128, E]))
                        be2_rep = pj.tile([128, E], F32)
                        nc.sync.dma_start(be2_rep[:], be2_d[0:1, :].to_broadcast([128, E]))
                        for tt in range(NT):
                            _layer_norm(
                                nc, lt2, h2_sb[:, tt, :], x2_sb[:, tt, :], g2_rep[:], be2_rep[:], "ln2"
                            )

                    # ---------- P9-P11: FFN (float32r) ----------
                    if upto < 9:
                        return
                    with tc.tile_pool(name="ffn", bufs=1) as fp:
                        h2T = fp.tile([128, ET, TOWN], BF16)
                        with tc.tile_pool(name="pst2", bufs=2, space="PSUM") as pst2:
                            for tt in range(NT):
                                for et in range(ET):
                                    ps = pst2.tile([128, 128], F32, tag="tr2")
                                    nc.tensor.transpose(
                                        ps[:], h2_sb[:, tt, 128 * et : 128 * (et + 1)], ident[:]
                                    )
                                    nc.vector.tensor_copy(
                                        h2T[:, et, 128 * tt : 128 * (tt + 1)], ps[:]
                                    )
                        b1_sb = fp.tile([128, FT], F32)
                        nc.sync.dma_start(b1_sb[:], b1_d.rearrange("(ft p) -> p ft", p=128))
                        aT = fp.tile([128, FT, TOWN], BF16)
                        with tc.tile_pool(name="pf", bufs=3, space="PSUM") as pf:
                            for ft in range(FT if upto >= 10 else 0):
                                w1t = w1p.tile([128, ET, 128], BF16, tag="w1")
                                nc.sync.dma_start(
                                    w1t[:],
                                    w1_d[128 * ft : 128 * (ft + 1), :].rearrange(
                                        "p (kt m) -> p kt m", kt=ET
                                    ),
                                )
                                ps = pf.tile([128, 512], F32, tag="f")
                                for kt in range(ET):
                                    nc.tensor.matmul(
                                        ps[:], w1t[:, kt, :], h2T[:, kt, :],
                                        start=(kt == 0), stop=(kt == ET - 1),
                                    )
                                # relu(ps + b1) on DVE, rounded to f32r
                                nc.vector.tensor_scalar(
                                    aT[:, ft, :], ps[:], b1_sb[:, ft : ft + 1], 0.0,
                                    op0=AluOpType.add, op1=AluOpType.max,
                                )
                        out_sb = h_sb
                        b2_rep = fp.tile([128, E], F32)
                        nc.sync.dma_start(b2_rep[:], b2_d[0:1, :].to_broadcast([128, E]))
                        for tt in range(NT):
                            nc.vector.tensor_tensor(
                                h2_sb[:, tt, :], h2_sb[:, tt, :], b2_rep[:], op=AluOpType.add
                            )
                        if upto < 11:
                            return
                        with tc.tile_pool(name="pff", bufs=8, space="PSUM") as pff:
                            accs = [pff.tile([128, 512], F32, tag="acc", name=f"acc{i}") for i in range(8)]
                            for ktf in range(FT):
                                w2t = w2p.tile([128, E], BF16, tag="w2")
                                nc.sync.dma_start(w2t[:], w2_d[128 * ktf : 128 * (ktf + 1), :])
                                for tt in range(NT):
                                    for nh in range(2):
                                        nc.tensor.matmul(
                                            accs[2 * tt + nh][:],
                                            aT[:, ktf, 128 * tt : 128 * (tt + 1)],
                                            w2t[:, 512 * nh : 512 * (nh + 1)],
                                            start=(ktf == 0), stop=(ktf == FT - 1),
                                        )
                            for tt in range(NT):
                                for nh in range(2):
                                    sl = slice(512 * nh, 512 * (nh + 1))
                                    nc.vector.tensor_tensor(
                                        out_sb[:, tt, sl], accs[2 * tt + nh][:],
                                        h2_sb[:, tt, sl], op=AluOpType.add,
                                    )
                        nc.sync.dma_start(out_d.rearrange("(tt p) e -> p tt e", p=128), out_sb[:])

                _run()
                _unwind()


            _emit()
    nc.compile()
    return nc


def _in_maps(inputs):
    x = np.asarray(inputs["x"], np.float32)
    wq = np.asarray(inputs["wq"], np.float32)
    wk = np.asarray(inputs["wk"], np.float32)
    wv = np.asarray(inputs["wv"], np.float32)
    w_proj = np.asarray(inputs["w_proj"], np.float32)
    w1 = np.asarray(inputs["w1"], np.float32)
    w1 = np.ascontiguousarray(
        w1.reshape(ET, 128, FT, 128).transpose(2, 1, 0, 3).reshape(DFF, E)
    ).astype(ml_dtypes.bfloat16)
    w2 = np.ascontiguousarray(np.asarray(inputs["w2"], np.float32)).astype(
        ml_dtypes.bfloat16
    )
    bp = np.asarray(inputs["b_proj"], np.float32).reshape(1, E)
    b1 = np.ascontiguousarray(np.asarray(inputs["b1"], np.float32))
    b2 = np.asarray(inputs["b2"], np.float32).reshape(1, E)
    g1 = np.asarray(inputs["gamma1"], np.float32).reshape(1, E)
    be1 = np.asarray(inputs["beta1"], np.float32).reshape(1, E)
    g2 = np.asarray(inputs["gamma2"], np.float32).reshape(1, E)
    be2 = np.asarray(inputs["beta2"], np.float32).reshape(1, E)
    wpe = []
    for b in range(B):
        m = np.zeros((2 * E, E), np.float32)
        for s_ in range(NCORES):
            if s_ // TP == b:
                lr = s_ % TP
                m[256 * s_ : 256 * (s_ + 1)] = w_proj[256 * lr : 256 * (lr + 1)]
        wpe.append(m.astype(ml_dtypes.bfloat16))
    ident = np.eye(128, dtype=np.float32)
    # mask_diag[p, 512*m + c] = 1 if p <= c - 128*m  (diagonal quad causal mask)
    md = np.zeros((128, 4, 512), np.float32)
    for m in range(4):
        for pp_ in range(128):
            cstart = 128 * m + pp_
            if cstart < 512:
                md[pp_, m, cstart:] = 1.0
    mask_diag = md.reshape(128, 2048).astype(ml_dtypes.bfloat16)

    maps = []
    for c in range(NCORES):
        b, j = c // TP, c % TP
        heads = slice(HL * j, HL * (j + 1))
        maps.append({
            "x_own": np.ascontiguousarray(x[b, TOWN * j : TOWN * (j + 1)]),
            "wq_s": np.ascontiguousarray(wq[heads].transpose(1, 0, 2).reshape(E, HL * HD)).astype(ml_dtypes.bfloat16),
            "wk_s": np.ascontiguousarray(wk[heads].transpose(1, 0, 2).reshape(E, HL * HD)).astype(ml_dtypes.bfloat16),
            "wv_s": np.ascontiguousarray(wv[heads].transpose(1, 0, 2).reshape(E, HL * HD)).astype(ml_dtypes.bfloat16),
            "w_proj": wpe[b], "w1": w1, "w2": w2,
            "b_proj": bp, "b1": b1, "b2": b2,
            "gamma1": g1, "beta1": be1, "gamma2": g2, "beta2": be2,
            "ident": ident, "mask_diag": mask_diag,
        })
    return maps


def kernel(**inputs) -> np.ndarray:
    if "nc" not in _CACHE:
        _CACHE["nc"] = build()
    nc = _CACHE["nc"]
    res = bass_utils.run_bass_kernel_spmd(
        nc, _in_maps(inputs), core_ids=list(range(NCORES))
    )
    out = np.empty((B, T, E), np.float32)
    for c in range(NCORES):
        b, j = c // TP, c % TP
        out[b, TOWN * j : TOWN * (j + 1)] = res.results[c]["out_own"]
    return out



# revision 18
# speedup vs baseline: 1.2895x; 1.1858x over previous
"""Trainium2 Bass kernel for a causal pre-LN decoder block (B=2, T=2048, E=1024,
H=16, hd=64, dff=4096), SPMD over 8 NeuronCores.

Sharding: batch split across the two 4-core groups (cores 0-3 -> batch 0,
cores 4-7 -> batch 1). Within a group, attention is tensor-parallel over heads
(4 heads per core, full sequence); LN, residuals and the FFN are
sequence-parallel (512 tokens per core). Collectives: an AllGather of z^T
(pre-gamma LN output, 2 pipelined rounds of 256 token columns) and an in-group
ReduceScatter(add) of per-core partial attention-output projections (each core
contracts only its own 256 o^T rows against its w_proj row slice, so the
projection matmul does no cross-batch waste).

LayerNorm runs mostly on the Activation engine (Square pass with accumulate
for var, Identity pass with per-token scale/bias for the normalize); gamma1 /
gamma2 are folded into wq/wk/wv/w1 on the host, beta1/beta2 into host-computed
biases, so the QKV and FFN matmuls consume the un-affine z directly.

QKV is interleaved with attention per query block (rank r's QKV chains, then
attention for query block r across all 4 local heads, then the partial
projection of token block r), which keeps PE busy while the Activation engine
works through the softmax Exp backlog.
"""

import numpy as np
import ml_dtypes

import concourse.bacc as bacc
import concourse.mybir as mybir
import concourse.tile as tile
from concourse import bass_utils
from concourse.alu_op_type import AluOpType
from concourse.mybir import ActivationFunctionType as AFT
from bass_rust import AxisListType

B, T, E, H, HD, DFF = 2, 2048, 1024, 16, 64, 4096
NCORES, TP = 8, 4
TOWN = T // TP        # 512 tokens owned per core
NT = TOWN // 128      # 4 own token tiles
ET = E // 128         # 8 tiles along E
KT = T // 128         # 16 kv tiles over full T
HL = H // TP          # 4 local heads
FT = DFF // 128       # 32 tiles along dff
EPS = 1e-5

F32 = mybir.dt.float32
BF16 = mybir.dt.bfloat16
RG = [[0, 1, 2, 3], [4, 5, 6, 7]]

_CACHE = {}


def build(single=False, upto=99):
    ndev = 1 if single else NCORES
    nc = bacc.Bacc("TRN2", target_bir_lowering=False, debug=False, num_devices=ndev)

    def din(name, shape, dt):
        return nc.dram_tensor(name, shape, dt, kind="ExternalInput").ap()

    x_d = din("x_own", [TOWN, E], F32)
    wq_d = din("wq_s", [E, HL * HD], BF16)   # gamma1-folded
    wk_d = din("wk_s", [E, HL * HD], BF16)
    wv_d = din("wv_s", [E, HL * HD], BF16)
    qb_d = din("qb_s", [128, 2], F32)        # beta1 @ wq, per (partition, mt)
    kb_d = din("kb_s", [128, 2], F32)
    vb_d = din("vb_s", [128, HL * HD], F32)  # beta1 @ wv, replicated over partitions
    wp_d = din("wp_s", [128, 2 * E], BF16)   # my w_proj rows, [p, (mt, e)]
    w1_d = din("w1", [DFF, E], BF16)         # gamma2-folded, host-reordered
    w2_d = din("w2", [DFF, E], BF16)
    b1_d = din("b1a", [DFF], F32)            # b1 + beta2 @ w1
    g1_d = din("gamma1", [1, E], F32)
    bb1_d = din("be1bp", [1, E], F32)        # beta1 + b_proj
    g2_d = din("gamma2", [1, E], F32)
    bb2_d = din("be2b2", [1, E], F32)        # beta2 + b2
    id_d = din("ident", [128, 128], F32)
    mk_d = din("mask_diag", [128, 4 * 512], BF16)
    out_d = nc.dram_tensor("out_own", [TOWN, E], F32, kind="ExternalOutput").ap()

    with tile.TileContext(nc) as tc:
        with (
            tc.tile_pool(name="dram", bufs=1, space="DRAM") as dram,
            tc.tile_pool(name="pp", bufs=1) as pp,
            tc.tile_pool(name="lns", bufs=2) as lns,
            tc.tile_pool(name="stg", bufs=2) as stg,
            tc.tile_pool(name="w1s", bufs=4) as w1p,
            tc.tile_pool(name="w2s", bufs=6) as w2p,
        ):
            ag_in = [dram.tile([E, 256], BF16, name=f"agi{g}") for g in range(2)]
            ag_out = [dram.tile([TP * E, 256], BF16, name=f"ago{g}") for g in range(2)]
            rs_in = dram.tile([T, E], BF16, name="rsi")
            rs_out = dram.tile([TOWN, E], BF16, name="rso")

            # ---- persistent SBUF ----
            ident = pp.tile([128, 128], F32)
            qb_sb = pp.tile([128, 2], F32)
            kb_sb = pp.tile([128, 2], F32)
            vb_sb = pp.tile([128, HL * HD], F32)
            z_sb = pp.tile([128, NT, E], F32)    # z1 -> h_pb -> x2 -> out staging
            z2_sb = pp.tile([128, NT, E], F32)   # LN1 scratch -> z2 -> h2b
            g1_rep = pp.tile([128, E], F32)
            bb1_rep = pp.tile([128, E], F32)
            b1_sb = pp.tile([128, FT], F32)

            def ln_core(xin, z_out, scr, tag):
                """z_out = (xin - mean)/sqrt(var+eps); scr is a [128,E] scratch."""
                s = lns.tile([128, 1], F32, tag=tag + "s")
                nc.vector.reduce_sum(s[:], xin, AxisListType.X)
                s2 = lns.tile([128, 1], F32, tag=tag + "s2")
                nc.scalar.activation(scr, xin, AFT.Square, accum_out=s2[:])
                m = lns.tile([128, 1], F32, tag=tag + "m")
                nc.vector.tensor_scalar(m[:], s[:], 1.0 / E, None, op0=AluOpType.mult)
                t = lns.tile([128, 1], F32, tag=tag + "t")
                nc.vector.tensor_scalar(
                    t[:], s2[:], 1.0 / E, EPS, op0=AluOpType.mult, op1=AluOpType.add
                )
                mm = lns.tile([128, 1], F32, tag=tag + "mm")
                nc.vector.tensor_tensor(mm[:], m[:], m[:], op=AluOpType.mult)
                veps = lns.tile([128, 1], F32, tag=tag + "ve")
                nc.vector.tensor_tensor(veps[:], t[:], mm[:], op=AluOpType.subtract)
                rv = lns.tile([128, 1], F32, tag=tag + "rv")
                nc.vector.reciprocal(rv[:], veps[:])
                rstd = lns.tile([128, 1], F32, tag=tag + "rs")
                nc.scalar.activation(rstd[:], rv[:], AFT.Sqrt)
                nmr = lns.tile([128, 1], F32, tag=tag + "nm")
                nc.vector.tensor_scalar(
                    nmr[:], m[:], rstd[:], -1.0, op0=AluOpType.mult, op1=AluOpType.mult
                )
                nc.scalar.activation(z_out, xin, AFT.Identity, bias=nmr[:], scale=rstd[:])

            def _phase_attn():
                with (
                    tc.tile_pool(name="att", bufs=1) as at,
                    tc.tile_pool(name="up", bufs=2) as up,
                ):
                    zT_full = at.tile([128, ET, TP, TOWN], BF16)
                    qT = at.tile([128, 2, T], BF16)
                    kT = at.tile([128, 2, T], BF16)
                    v_aug = at.tile([128, KT, HL, HD + 1], BF16)
                    wq_sb = at.tile([128, ET, HL * HD], BF16)
                    wk_sb = at.tile([128, ET, HL * HD], BF16)
                    wv_sb = at.tile([128, ET, HL * HD], BF16)
                    wp_sb = at.tile([128, 2, E], BF16)
                    mask = at.tile([128, 4 * 512], BF16)

                    # ---- x loads + small weights ----
                    with tc.tile_pool(name="xp", bufs=4) as xp:
                        xts = []
                        for tt in range(NT):
                            xt = xp.tile([128, E], F32, tag="xt")
                            nc.sync.dma_start(xt[:], x_d[128 * tt : 128 * (tt + 1), :])
                            xts.append(xt)
                        nc.sync.dma_start(ident[:], id_d[:])
                        nc.sync.dma_start(qb_sb[:], qb_d[:])
                        nc.sync.dma_start(kb_sb[:], kb_d[:])
                        nc.sync.dma_start(
                            wq_sb[:], wq_d.rearrange("(kt p) m -> p kt m", p=128)
                        )
                        nc.sync.dma_start(
                            wk_sb[:], wk_d.rearrange("(kt p) m -> p kt m", p=128)
                        )

                        # ---- LN1 ----
                        if upto >= 1:
                            for tt in range(NT):
                                ln_core(xts[tt][:], z_sb[:, tt, :], z2_sb[:, tt, :], "l1")

                        # ---- transpose z + AllGather rounds ----
                        if upto >= 2:
                            with (
                                tc.tile_pool(name="pst", bufs=2, space="PSUM") as pst,
                                tc.tile_pool(name="ztp", bufs=1) as ztp,
                            ):
                                for g in range(2):
                                    zTo = ztp.tile([128, ET, 256], BF16, tag="zTo")
                                    for lt in range(2):
                                        tt = 2 * g + lt
                                        for et in range(ET):
                                            ps = pst.tile([128, 128], F32, tag="tr")
                                            nc.tensor.transpose(
                                                ps[:],
                                                z_sb[:, tt, 128 * et : 128 * (et + 1)],
                                                ident[:],
                                            )
                                            nc.vector.tensor_copy(
                                                zTo[:, et, 128 * lt : 128 * (lt + 1)], ps[:]
                                            )
                                    nc.sync.dma_start(
                                        ag_in[g].rearrange("(et p) t -> p et t", p=128),
                                        zTo[:],
                                    )
                                    if single:
                                        for r in range(TP):
                                            nc.sync.dma_start(
                                                ag_out[g][E * r : E * (r + 1), :], ag_in[g][:]
                                            )
                                    else:
                                        nc.gpsimd.collective_compute(
                                            "AllGather", AluOpType.bypass, replica_groups=RG,
                                            ins=[ag_in[g].opt()], outs=[ag_out[g].opt()],
                                        )
                                    for r in range(TP):
                                        nc.sync.dma_start(
                                            zT_full[:, :, r, 256 * g : 256 * (g + 1)],
                                            ag_out[g][E * r : E * (r + 1), :].rearrange(
                                                "(et p) t -> p et t", p=128
                                            ),
                                        )

                    # ---- remaining weight DMAs ----
                    nc.sync.dma_start(wv_sb[:], wv_d.rearrange("(kt p) m -> p kt m", p=128))
                    nc.sync.dma_start(vb_sb[:], vb_d[:])
                    nc.sync.dma_start(wp_sb[:], wp_d.rearrange("p (mt e) -> p mt e", mt=2))
                    nc.sync.dma_start(mask[:], mk_d[:])
                    nc.sync.dma_start(g1_rep[:], g1_d[0:1, :].to_broadcast([128, E]))
                    nc.sync.dma_start(bb1_rep[:], bb1_d[0:1, :].to_broadcast([128, E]))
                    nc.vector.memset(v_aug[:, :, :, HD], 1.0)

                    if upto < 4:
                        return

                    # ---- merged QKV + attention, per rank/query-block r ----
                    with (
                        tc.tile_pool(name="pq", bufs=2, space="PSUM") as pq,
                        tc.tile_pool(name="pss", bufs=2, space="PSUM") as pss,
                        tc.tile_pool(name="pso", bufs=2, space="PSUM") as pso,
                    ):
                        def qkv_rank(r):
                            for g in range(2):
                                cs = 512 * r + 256 * g
                                for dstT, w_sb, bias, eng in (
                                    (qT, wq_sb, qb_sb, "act"),
                                    (kT, wk_sb, kb_sb, "dve"),
                                ):
                                    for mt in range(2):
                                        ps = pq.tile([128, 256], F32, tag="qk")
                                        for kt in range(ET):
                                            nc.tensor.matmul(
                                                ps[:],
                                                w_sb[:, kt, 128 * mt : 128 * (mt + 1)],
                                                zT_full[:, kt, r, 256 * g : 256 * (g + 1)],
                                                start=(kt == 0), stop=(kt == ET - 1),
                                            )
                                        dsl = dstT[:, mt, cs : cs + 256]
                                        if eng == "act":
                                            nc.scalar.activation(
                                                dsl, ps[:], AFT.Identity,
                                                bias=bias[:, mt : mt + 1],
                                            )
                                        else:
                                            nc.vector.tensor_scalar(
                                                dsl, ps[:], bias[:, mt : mt + 1], None,
                                                op0=AluOpType.add,
                                            )
                                for lt in range(2):
                                    t16 = 4 * r + 2 * g + lt
                                    ps = pq.tile([128, 256], F32, tag="qk")
                                    for kt in range(ET):
                                        nc.tensor.matmul(
                                            ps[:],
                                            zT_full[
                                                :, kt, r,
                                                256 * g + 128 * lt : 256 * g + 128 * (lt + 1),
                                            ],
                                            wv_sb[:, kt, :],
                                            start=(kt == 0), stop=(kt == ET - 1),
                                        )
                                    nc.vector.tensor_tensor(
                                        v_aug[:, t16, :, 0:HD],
                                        ps[:].rearrange("p (hh d) -> p hh d", d=HD),
                                        vb_sb[:].rearrange("p (hh d) -> p hh d", d=HD),
                                        op=AluOpType.add,
                                    )

                        def att_scores(hh, qb):
                            pb = 64 * (hh % 2)
                            mt = hh // 2
                            nkv = 4 * qb + 4
                            u = up.tile([128, KT, 512], BF16, tag="u")
                            jt0 = 0
                            while jt0 < nkv:
                                gw = min(2, nkv - jt0)
                                ps = pss.tile([128, 2, 512], F32, tag="s")
                                for m_ in range(gw):
                                    jt = jt0 + m_
                                    co = max(0, 128 * (jt - 4 * qb))
                                    nc.tensor.matmul(
                                        ps[:, m_, co:512],
                                        kT[pb : pb + 64, mt, 128 * jt : 128 * (jt + 1)],
                                        qT[pb : pb + 64, mt, 512 * qb + co : 512 * (qb + 1)],
                                        start=True, stop=True,
                                    )
                                if 128 * (jt0 + gw - 1 - 4 * qb) <= 0:
                                    nc.scalar.activation(
                                        u[:, jt0 : jt0 + gw, :], ps[:, 0:gw, :],
                                        AFT.Exp, scale=1.0 / np.sqrt(HD),
                                    )
                                else:
                                    for m_ in range(gw):
                                        jt = jt0 + m_
                                        co = max(0, 128 * (jt - 4 * qb))
                                        nc.scalar.activation(
                                            u[:, jt, co:512], ps[:, m_, co:512],
                                            AFT.Exp, scale=1.0 / np.sqrt(HD),
                                        )
                                        if co > 0:
                                            nc.vector.memset(u[:, jt, 0:co], 0.0)
                                jt0 += gw
                            nc.vector.tensor_tensor(
                                u[:, 4 * qb : 4 * qb + 4, :],
                                u[:, 4 * qb : 4 * qb + 4, :],
                                mask[:], op=AluOpType.mult,
                            )
                            return u

                        def att_pv(hh, qb, u, o_blk):
                            pb = 64 * (hh % 2)
                            mt = hh // 2
                            nkv = 4 * qb + 4
                            po = pso.tile([128, 512], F32, tag="o")
                            for jt in range(nkv):
                                nc.tensor.matmul(
                                    po[0 : HD + 1, :],
                                    v_aug[:, jt, hh, :],
                                    u[:, jt, :],
                                    start=(jt == 0), stop=(jt == nkv - 1),
                                )
                            rz = stg.tile([1, 512], F32, tag="rz")
                            nc.vector.reciprocal(rz[:], po[HD : HD + 1, :])
                            rz_rep = stg.tile([64, 512], F32, tag="rzr")
                            nc.gpsimd.partition_broadcast(rz_rep[:], rz[:])
                            nc.vector.tensor_tensor(
                                o_blk[pb : pb + 64, mt, :],
                                po[0:HD, :], rz_rep[:], op=AluOpType.mult,
                            )

                        def proj_block(r, o_blk):
                            for t4 in range(4):
                                tsl = slice(512 * r + 128 * t4, 512 * r + 128 * (t4 + 1))
                                for nh in range(2):
                                    psj = pso.tile([128, 512], F32, tag="o")
                                    for mt in range(2):
                                        nc.tensor.matmul(
                                            psj[:],
                                            o_blk[:, mt, 128 * t4 : 128 * (t4 + 1)],
                                            wp_sb[:, mt, 512 * nh : 512 * (nh + 1)],
                                            start=(mt == 0), stop=(mt == 1),
                                        )
                                    st = stg.tile([128, 512], BF16, tag="st")
                                    nc.vector.tensor_copy(st[:], psj[:])
                                    nc.sync.dma_start(
                                        rs_in[tsl, 512 * nh : 512 * (nh + 1)], st[:]
                                    )

                        for r in range(TP):
                            qkv_rank(r)
                            if upto < 5:
                                continue
                            o_blk = up.tile([128, 2, 512], BF16, tag="ob")
                            prev = None
                            for hh in range(HL):
                                u = att_scores(hh, r)
                                if prev is not None:
                                    att_pv(prev[0], r, prev[1], o_blk)
                                prev = (hh, u)
                            att_pv(prev[0], r, prev[1], o_blk)
                            if upto >= 6:
                                proj_block(r, o_blk)
                            # h_pb for one token tile per iteration (off critical path)
                            nc.vector.tensor_tensor(
                                z_sb[:, r, :], z_sb[:, r, :], g1_rep[:], op=AluOpType.mult
                            )
                            nc.vector.tensor_tensor(
                                z_sb[:, r, :], z_sb[:, r, :], bb1_rep[:], op=AluOpType.add
                            )

                    if upto < 6:
                        return
                    # ---- ReduceScatter of partial projections ----
                    if single:
                        nc.sync.dma_start(rs_out[:], rs_in[0:TOWN, :])
                    else:
                        nc.gpsimd.collective_compute(
                            "ReduceScatter", AluOpType.add, replica_groups=RG,
                            ins=[rs_in.opt()], outs=[rs_out.opt()],
                        )

            _phase_attn()

            # ---- x2 = h_pb + attn_out; LN2; transpose z2 ----
            if upto >= 7:
                pop_cm = tc.tile_pool(name="post", bufs=1)
                pop = pop_cm.__enter__()
                x2s = pop.tile([128, NT, E], BF16)
                z2T = pop.tile([128, ET, TOWN], BF16)
                g2_rep = pop.tile([128, E], F32)
                bb2_rep = pop.tile([128, E], F32)
                nc.sync.dma_start(
                    x2s[:], rs_out.rearrange("(tt p) e -> p tt e", p=128)
                )
                nc.sync.dma_start(g2_rep[:], g2_d[0:1, :].to_broadcast([128, E]))
                nc.sync.dma_start(bb2_rep[:], bb2_d[0:1, :].to_broadcast([128, E]))
                with tc.tile_pool(name="pst2", bufs=2, space="PSUM") as pst2:
                    for tt in range(NT):
                        nc.vector.tensor_tensor(
                            z_sb[:, tt, :], z_sb[:, tt, :], x2s[:, tt, :], op=AluOpType.add
                        )
                        ln_core(z_sb[:, tt, :], z2_sb[:, tt, :], x2s[:, tt, :], "l2")
                        for et in range(ET):
                            ps = pst2.tile([128, 128], F32, tag="tr2")
                            nc.tensor.transpose(
                                ps[:], z2_sb[:, tt, 128 * et : 128 * (et + 1)], ident[:]
                            )
                            nc.vector.tensor_copy(
                                z2T[:, et, 128 * tt : 128 * (tt + 1)], ps[:]
                            )

            # ---- FFN ----
            if upto >= 9:
                with tc.tile_pool(name="ffp", bufs=1) as fp:
                    nc.sync.dma_start(b1_sb[:], b1_d.rearrange("(ft p) -> p ft", p=128))
                    aT = fp.tile([128, FT, TOWN], BF16)
                    with tc.tile_pool(name="pf", bufs=3, space="PSUM") as pf:
                        for ft in range(FT if upto >= 10 else 0):
                            w1t = w1p.tile([128, ET, 128], BF16, tag="w1")
                            nc.sync.dma_start(
                                w1t[:],
                                w1_d[128 * ft : 128 * (ft + 1), :].rearrange(
                                    "p (kt m) -> p kt m", kt=ET
                                ),
                            )
                            ps = pf.tile([128, 512], F32, tag="f")
                            for kt in range(ET):
                                nc.tensor.matmul(
                                    ps[:], w1t[:, kt, :], z2T[:, kt, :],
                                    start=(kt == 0), stop=(kt == ET - 1),
                                )
                            nc.scalar.activation(
                                aT[:, ft, :], ps[:], AFT.Relu, bias=b1_sb[:, ft : ft + 1]
                            )
                            if ft == 0:
                                # h2b = z2 * g2 + (beta2 + b2), off critical path
                                for tt in range(NT):
                                    nc.vector.tensor_tensor(
                                        z2_sb[:, tt, :], z2_sb[:, tt, :], g2_rep[:],
                                        op=AluOpType.mult,
                                    )
                                    nc.vector.tensor_tensor(
                                        z2_sb[:, tt, :], z2_sb[:, tt, :], bb2_rep[:],
                                        op=AluOpType.add,
                                    )
                    if upto >= 11:
                        with tc.tile_pool(name="pff", bufs=8, space="PSUM") as pff:
                            accs = [
                                pff.tile([128, 512], F32, tag="acc", name=f"acc{i}")
                                for i in range(8)
                            ]
                            for ktf in range(FT):
                                w2t = w2p.tile([128, E], BF16, tag="w2")
                                nc.sync.dma_start(
                                    w2t[:], w2_d[128 * ktf : 128 * (ktf + 1), :]
                                )
                                for tt in range(NT):
                                    for nh in range(2):
                                        nc.tensor.matmul(
                                            accs[2 * tt + nh][:],
                                            aT[:, ktf, 128 * tt : 128 * (tt + 1)],
                                            w2t[:, 512 * nh : 512 * (nh + 1)],
                                            start=(ktf == 0), stop=(ktf == FT - 1),
                                        )
                            for tt in range(NT):
                                for nh in range(2):
                                    sl = slice(512 * nh, 512 * (nh + 1))
                                    nc.vector.tensor_tensor(
                                        z_sb[:, tt, sl], accs[2 * tt + nh][:],
                                        z2_sb[:, tt, sl], op=AluOpType.add,
                                    )
                                nc.sync.dma_start(
                                    out_d[128 * tt : 128 * (tt + 1), :], z_sb[:, tt, :]
                                )
            if upto >= 7:
                pop_cm.__exit__(None, None, None)

    nc.compile()
    return nc


def _in_maps(inputs):
    bf16 = ml_dtypes.bfloat16
    x = np.asarray(inputs["x"], np.float32)
    wq = np.asarray(inputs["wq"], np.float32)
    wk = np.asarray(inputs["wk"], np.float32)
    wv = np.asarray(inputs["wv"], np.float32)
    w_proj = np.asarray(inputs["w_proj"], np.float32)
    b_proj = np.asarray(inputs["b_proj"], np.float32)
    g1 = np.asarray(inputs["gamma1"], np.float32)
    be1 = np.asarray(inputs["beta1"], np.float32)
    g2 = np.asarray(inputs["gamma2"], np.float32)
    be2 = np.asarray(inputs["beta2"], np.float32)
    w1 = np.asarray(inputs["w1"], np.float32)
    b1 = np.asarray(inputs["b1"], np.float32)
    w2 = np.asarray(inputs["w2"], np.float32)
    b2 = np.asarray(inputs["b2"], np.float32)

    wq_f = g1[None, :, None] * wq   # [H, E, hd]
    wk_f = g1[None, :, None] * wk
    wv_f = g1[None, :, None] * wv
    w1_f = g2[:, None] * w1         # [E, dff]
    qb_full = np.einsum("e,hek->hk", be1, wq)   # [H, hd]
    kb_full = np.einsum("e,hek->hk", be1, wk)
    vb_full = np.einsum("e,hek->hk", be1, wv)
    b1a = b1 + be2 @ w1
    be1bp = (be1 + b_proj).reshape(1, E)
    be2b2 = (be2 + b2).reshape(1, E)

    w1r = np.ascontiguousarray(
        w1_f.reshape(ET, 128, FT, 128).transpose(2, 1, 0, 3).reshape(DFF, E)
    ).astype(bf16)
    w2c = np.ascontiguousarray(w2).astype(bf16)
    ident = np.eye(128, dtype=np.float32)
    md = np.zeros((128, 4, 512), np.float32)
    for m in range(4):
        for p_ in range(128):
            cstart = 128 * m + p_
            if cstart < 512:
                md[p_, m, cstart:] = 1.0
    mask_diag = md.reshape(128, 2048).astype(bf16)

    pidx = np.arange(128)
    maps = []
    for c in range(NCORES):
        b, j = c // TP, c % TP
        heads = slice(HL * j, HL * (j + 1))
        qb_s = np.stack(
            [qb_full[heads][2 * mt + pidx // 64, pidx % 64] for mt in range(2)], axis=1
        ).astype(np.float32)
        kb_s = np.stack(
            [kb_full[heads][2 * mt + pidx // 64, pidx % 64] for mt in range(2)], axis=1
        ).astype(np.float32)
        vb_s = np.broadcast_to(
            vb_full[heads].reshape(1, HL * HD), (128, HL * HD)
        ).astype(np.float32)
        wp_rows = np.stack(
            [w_proj[64 * (HL * j + 2 * mt + pidx // 64) + pidx % 64, :] for mt in range(2)],
            axis=1,
        )  # [128, 2, E]
        wp_s = np.ascontiguousarray(wp_rows.reshape(128, 2 * E)).astype(bf16)

        maps.append({
            "x_own": np.ascontiguousarray(x[b, TOWN * j : TOWN * (j + 1)]),
            "wq_s": np.ascontiguousarray(
                wq_f[heads].transpose(1, 0, 2).reshape(E, HL * HD)).astype(bf16),
            "wk_s": np.ascontiguousarray(
                wk_f[heads].transpose(1, 0, 2).reshape(E, HL * HD)).astype(bf16),
            "wv_s": np.ascontiguousarray(
                wv_f[heads].transpose(1, 0, 2).reshape(E, HL * HD)).astype(bf16),
            "qb_s": qb_s, "kb_s": kb_s, "vb_s": np.ascontiguousarray(vb_s),
            "wp_s": wp_s,
            "w1": w1r, "w2": w2c, "b1a": np.ascontiguousarray(b1a),
            "gamma1": g1.reshape(1, E), "be1bp": be1bp,
            "gamma2": g2.reshape(1, E), "be2b2": be2b2,
            "ident": ident, "mask_diag": mask_diag,
        })
    return maps


def kernel(**inputs) -> np.ndarray:
    if "nc" not in _CACHE:
        _CACHE["nc"] = build()
    nc = _CACHE["nc"]
    res = bass_utils.run_bass_kernel_spmd(
        nc, _in_maps(inputs), core_ids=list(range(NCORES))
    )
    out = np.empty((B, T, E), np.float32)
    for c in range(NCORES):
        b, j = c // TP, c % TP
        out[b, TOWN * j : TOWN * (j + 1)] = res.results[c]["out_own"]
    return out


# revision 31
# speedup vs baseline: 1.3738x; 1.0654x over previous
"""Trainium2 Bass kernel for a causal pre-LN decoder block (B=2, T=2048, E=1024,
H=16, hd=64, dff=4096), SPMD over 8 NeuronCores.

Sharding: batch split across the two 4-core groups (cores 0-3 -> batch 0,
cores 4-7 -> batch 1). Within a group, attention is tensor-parallel over heads
(4 heads per core, full sequence); LN, residuals and the FFN are
sequence-parallel (512 tokens per core). Collectives: an AllGather of z^T
(pre-gamma LN output, 2 pipelined rounds of 256 token columns) and an in-group
ReduceScatter(add) of per-core partial attention-output projections (each core
contracts only its own 256 o^T rows against its w_proj row slice, so the
projection matmul does no cross-batch waste).

LayerNorm runs mostly on the Activation engine (Square pass with accumulate
for var, Identity pass with per-token scale/bias for the normalize); gamma1 /
gamma2 are folded into wq/wk/wv/w1 on the host, beta1/beta2 into host-computed
biases, so the QKV and FFN matmuls consume the un-affine z directly.

QKV is interleaved with attention per query block (rank r's QKV chains, then
attention for query block r across all 4 local heads, then the partial
projection of token block r), which keeps PE busy while the Activation engine
works through the softmax Exp backlog.
"""

import numpy as np
import ml_dtypes

import concourse.bacc as bacc
import concourse.mybir as mybir
import concourse.tile as tile
from concourse import bass_utils
from concourse.alu_op_type import AluOpType
from concourse.mybir import ActivationFunctionType as AFT
from bass_rust import AxisListType

B, T, E, H, HD, DFF = 2, 2048, 1024, 16, 64, 4096
NCORES, TP = 8, 4
TOWN = T // TP        # 512 tokens owned per core
NT = TOWN // 128      # 4 own token tiles
ET = E // 128         # 8 tiles along E
KT = T // 128         # 16 kv tiles over full T
HL = H // TP          # 4 local heads
FT = DFF // 128       # 32 tiles along dff
EPS = 1e-5

F32 = mybir.dt.float32
BF16 = mybir.dt.bfloat16
RG = [[0, 1, 2, 3], [4, 5, 6, 7]]

_CACHE = {}


def build(single=False, upto=99):
    ndev = 1 if single else NCORES
    nc = bacc.Bacc("TRN2", target_bir_lowering=False, debug=False, num_devices=ndev)

    def din(name, shape, dt):
        return nc.dram_tensor(name, shape, dt, kind="ExternalInput").ap()

    x_d = din("x_own", [TOWN, E], F32)
    wq_d = din("wq_s", [E, HL * HD], BF16)   # gamma1-folded
    wk_d = din("wk_s", [E, HL * HD], BF16)
    wv_d = din("wv_s", [E, HL * HD], BF16)
    qb_d = din("qb_s", [128, 2], F32)        # beta1 @ wq, per (partition, mt)
    kb_d = din("kb_s", [128, 2], F32)
    vb_d = din("vb_s", [128, HL * HD], F32)  # beta1 @ wv, replicated over partitions
    wp_d = din("wp_s", [128, 2 * E], BF16)   # my w_proj rows, [p, (mt, e)]
    w1_d = din("w1", [DFF, E], BF16)         # gamma2-folded, host-reordered
    w2_d = din("w2", [DFF, E], BF16)
    b1_d = din("b1a", [DFF], F32)            # b1 + beta2 @ w1
    g1_d = din("gamma1", [1, E], F32)
    bb1_d = din("be1bp", [1, E], F32)        # beta1 + b_proj
    g2_d = din("gamma2", [1, E], F32)
    bb2_d = din("be2b2", [1, E], F32)        # beta2 + b2
    id_d = din("ident", [128, 128], F32)
    mk_d = din("mask_diag", [128, 4 * 512], BF16)
    out_d = nc.dram_tensor("out_own", [TOWN, E], F32, kind="ExternalOutput").ap()

    with tile.TileContext(nc) as tc:
        with (
            tc.tile_pool(name="dram", bufs=1, space="DRAM") as dram,
            tc.tile_pool(name="pp", bufs=1) as pp,
            tc.tile_pool(name="lns", bufs=2) as lns,
            tc.tile_pool(name="stg", bufs=2) as stg,
            tc.tile_pool(name="w1s", bufs=4) as w1p,
            tc.tile_pool(name="w2s", bufs=6) as w2p,
        ):
            ag_in = [dram.tile([E, 256], BF16, name=f"agi{g}") for g in range(2)]
            ag_out = [dram.tile([TP * E, 256], BF16, name=f"ago{g}") for g in range(2)]
            rs_in = dram.tile([T, E], BF16, name="rsi")
            rs_out = dram.tile([TOWN, E], BF16, name="rso")

            # ---- persistent SBUF ----
            ident = pp.tile([128, 128], F32)
            qb_sb = pp.tile([128, 2], F32)
            kb_sb = pp.tile([128, 2], F32)
            vb_sb = pp.tile([128, HL * HD], F32)
            z_sb = pp.tile([128, NT, E], F32)    # z1 -> h_pb -> x2 -> out staging
            z2_sb = pp.tile([128, NT, E], F32)   # LN1 scratch -> z2 -> h2b
            g1_rep = pp.tile([128, E], F32)
            bb1_rep = pp.tile([128, E], F32)
            b1_sb = pp.tile([128, FT], F32)

            def ln_core_dve(xin, z_out, scr, tag):
                """DVE-heavy LN (no gamma/beta): keeps the Act engine free."""
                s = lns.tile([128, 1], F32, tag=tag + "ds")
                nc.vector.reduce_sum(s[:], xin, AxisListType.X)
                s2 = lns.tile([128, 1], F32, tag=tag + "d2")
                nc.vector.tensor_tensor(scr, xin, xin, op=AluOpType.mult)
                nc.vector.reduce_sum(s2[:], scr, AxisListType.X)
                m = lns.tile([128, 1], F32, tag=tag + "dm")
                nc.vector.tensor_scalar(m[:], s[:], 1.0 / E, None, op0=AluOpType.mult)
                t = lns.tile([128, 1], F32, tag=tag + "dt")
                nc.vector.tensor_scalar(
                    t[:], s2[:], 1.0 / E, EPS, op0=AluOpType.mult, op1=AluOpType.add
                )
                mm = lns.tile([128, 1], F32, tag=tag + "dmm")
                nc.vector.tensor_tensor(mm[:], m[:], m[:], op=AluOpType.mult)
                veps = lns.tile([128, 1], F32, tag=tag + "de")
                nc.vector.tensor_tensor(veps[:], t[:], mm[:], op=AluOpType.subtract)
                rv = lns.tile([128, 1], F32, tag=tag + "dr")
                nc.vector.reciprocal(rv[:], veps[:])
                rstd = lns.tile([128, 1], F32, tag=tag + "dsr")
                nc.scalar.activation(rstd[:], rv[:], AFT.Sqrt)
                nmean = lns.tile([128, 1], F32, tag=tag + "dnm")
                nc.vector.tensor_scalar(nmean[:], m[:], -1.0, None, op0=AluOpType.mult)
                nc.vector.tensor_scalar(
                    z_out, xin, nmean[:], rstd[:], op0=AluOpType.add, op1=AluOpType.mult
                )

            def ln_core(xin, z_out, scr, tag):
                """z_out = (xin - mean)/sqrt(var+eps); scr is a [128,E] scratch."""
                s = lns.tile([128, 1], F32, tag=tag + "s")
                nc.vector.reduce_sum(s[:], xin, AxisListType.X)
                s2 = lns.tile([128, 1], F32, tag=tag + "s2")
                nc.scalar.activation(scr, xin, AFT.Square, accum_out=s2[:])
                m = lns.tile([128, 1], F32, tag=tag + "m")
                nc.vector.tensor_scalar(m[:], s[:], 1.0 / E, None, op0=AluOpType.mult)
                t = lns.tile([128, 1], F32, tag=tag + "t")
                nc.vector.tensor_scalar(
                    t[:], s2[:], 1.0 / E, EPS, op0=AluOpType.mult, op1=AluOpType.add
                )
                mm = lns.tile([128, 1], F32, tag=tag + "mm")
                nc.vector.tensor_tensor(mm[:], m[:], m[:], op=AluOpType.mult)
                veps = lns.tile([128, 1], F32, tag=tag + "ve")
                nc.vector.tensor_tensor(veps[:], t[:], mm[:], op=AluOpType.subtract)
                rv = lns.tile([128, 1], F32, tag=tag + "rv")
                nc.vector.reciprocal(rv[:], veps[:])
                rstd = lns.tile([128, 1], F32, tag=tag + "rs")
                nc.scalar.activation(rstd[:], rv[:], AFT.Sqrt)
                nmr = lns.tile([128, 1], F32, tag=tag + "nm")
                nc.vector.tensor_scalar(
                    nmr[:], m[:], rstd[:], -1.0, op0=AluOpType.mult, op1=AluOpType.mult
                )
                nc.scalar.activation(z_out, xin, AFT.Identity, bias=nmr[:], scale=rstd[:])

            def _phase_attn():
                with (
                    tc.tile_pool(name="att", bufs=1) as at,
                    tc.tile_pool(name="up", bufs=2) as up,
                ):
                    zT_full = at.tile([128, ET, TP, TOWN], BF16)
                    qT = at.tile([128, 2, T], BF16)
                    kT = at.tile([128, 2, T], BF16)
                    v_aug = at.tile([128, KT, HL, HD + 1], BF16)
                    wq_sb = at.tile([128, ET, HL * HD], BF16)
                    wk_sb = at.tile([128, ET, HL * HD], BF16)
                    wv_sb = at.tile([128, ET, HL * HD], BF16)
                    wp_sb = at.tile([128, 2, E], BF16)
                    mask = at.tile([128, 4 * 512], BF16)

                    # ---- x loads + small weights ----
                    with tc.tile_pool(name="xp", bufs=4) as xp:
                        xts = []
                        for tt in range(NT):
                            xt = xp.tile([128, E], F32, tag="xt")
                            nc.sync.dma_start(xt[:], x_d[128 * tt : 128 * (tt + 1), :])
                            xts.append(xt)
                        nc.sync.dma_start(ident[:], id_d[:])
                        nc.sync.dma_start(qb_sb[:], qb_d[:])
                        nc.sync.dma_start(kb_sb[:], kb_d[:])
                        nc.sync.dma_start(
                            wq_sb[:], wq_d.rearrange("(kt p) m -> p kt m", p=128)
                        )
                        nc.sync.dma_start(
                            wk_sb[:], wk_d.rearrange("(kt p) m -> p kt m", p=128)
                        )

                        # ---- LN1 ----
                        if upto >= 1:
                            for tt in range(NT):
                                fn = ln_core if tt % 2 == 0 else ln_core_dve
                                fn(xts[tt][:], z_sb[:, tt, :], z2_sb[:, tt, :], "l1")

                        # ---- transpose z + AllGather rounds ----
                        def unb(g, r):
                            nc.sync.dma_start(
                                zT_full[:, :, r, 256 * g : 256 * (g + 1)],
                                ag_out[g][E * r : E * (r + 1), :].rearrange(
                                    "(et p) t -> p et t", p=128
                                ),
                            )

                        if upto >= 2:
                            with (
                                tc.tile_pool(name="pst", bufs=2, space="PSUM") as pst,
                                tc.tile_pool(name="ztp", bufs=1) as ztp,
                            ):
                                for g in range(2):
                                    zTo = ztp.tile([128, ET, 256], BF16, tag="zTo")
                                    for lt in range(2):
                                        tt = 2 * g + lt
                                        for et in range(ET):
                                            ps = pst.tile([128, 128], F32, tag="tr")
                                            nc.tensor.transpose(
                                                ps[:],
                                                z_sb[:, tt, 128 * et : 128 * (et + 1)],
                                                ident[:],
                                            )
                                            nc.vector.tensor_copy(
                                                zTo[:, et, 128 * lt : 128 * (lt + 1)], ps[:]
                                            )
                                    nc.sync.dma_start(
                                        ag_in[g].rearrange("(et p) t -> p et t", p=128),
                                        zTo[:],
                                    )
                                    if single:
                                        for r in range(TP):
                                            nc.sync.dma_start(
                                                ag_out[g][E * r : E * (r + 1), :], ag_in[g][:]
                                            )
                                    else:
                                        nc.gpsimd.collective_compute(
                                            "AllGather", AluOpType.bypass, replica_groups=RG,
                                            ins=[ag_in[g].opt()], outs=[ag_out[g].opt()],
                                        )
                                    # unbounce r0 first, interleave weights by need time
                                    unb(g, 0)
                                    if g == 0:
                                        nc.sync.dma_start(
                                            wv_sb[:],
                                            wv_d.rearrange("(kt p) m -> p kt m", p=128),
                                        )
                                        nc.sync.dma_start(vb_sb[:], vb_d[:])
                                    else:
                                        nc.sync.dma_start(
                                            wp_sb[:],
                                            wp_d.rearrange("p (mt e) -> p mt e", mt=2),
                                        )
                                        nc.sync.dma_start(mask[:], mk_d[:])
                                    for r in range(1, TP):
                                        unb(g, r)

                    nc.sync.dma_start(g1_rep[:], g1_d[0:1, :].to_broadcast([128, E]))
                    nc.sync.dma_start(bb1_rep[:], bb1_d[0:1, :].to_broadcast([128, E]))
                    nc.vector.memset(v_aug[:, :, :, HD], 1.0)

                    if upto < 4:
                        return

                    # ---- merged QKV + attention, per rank/query-block r ----
                    with (
                        tc.tile_pool(name="pq", bufs=2, space="PSUM") as pq,
                        tc.tile_pool(name="pss", bufs=2, space="PSUM") as pss,
                        tc.tile_pool(name="pso", bufs=2, space="PSUM") as pso,
                    ):
                        def qkv_rank(r, g):
                            if True:
                                cs = 512 * r + 256 * g
                                for dstT, w_sb, bias, eng in (
                                    (qT, wq_sb, qb_sb, "dve"),
                                    (kT, wk_sb, kb_sb, "dve"),
                                ):
                                    for mt in range(2):
                                        ps = pq.tile([128, 256], F32, tag="qk")
                                        for kt in range(ET):
                                            nc.tensor.matmul(
                                                ps[:],
                                                w_sb[:, kt, 128 * mt : 128 * (mt + 1)],
                                                zT_full[:, kt, r, 256 * g : 256 * (g + 1)],
                                                start=(kt == 0), stop=(kt == ET - 1),
                                            )
                                        dsl = dstT[:, mt, cs : cs + 256]
                                        veng = nc.vector if eng == "dve" else nc.gpsimd
                                        veng.tensor_scalar(
                                            dsl, ps[:], bias[:, mt : mt + 1], None,
                                            op0=AluOpType.add,
                                        )
                                for lt in range(2):
                                    t16 = 4 * r + 2 * g + lt
                                    ps = pq.tile([128, 256], F32, tag="qk")
                                    for kt in range(ET):
                                        nc.tensor.matmul(
                                            ps[:],
                                            zT_full[
                                                :, kt, r,
                                                256 * g + 128 * lt : 256 * g + 128 * (lt + 1),
                                            ],
                                            wv_sb[:, kt, :],
                                            start=(kt == 0), stop=(kt == ET - 1),
                                        )
                                    nc.vector.tensor_tensor(
                                        v_aug[:, t16, :, 0:HD],
                                        ps[:].rearrange("p (hh d) -> p hh d", d=HD),
                                        vb_sb[:].rearrange("p (hh d) -> p hh d", d=HD),
                                        op=AluOpType.add,
                                    )

                        def att_scores(hh, qb):
                            pb = 64 * (hh % 2)
                            mt = hh // 2
                            nkv = 4 * qb + 4
                            u = up.tile([128, KT, 512], BF16, tag="u")
                            jt0 = 0
                            while jt0 < nkv:
                                gw = min(2, nkv - jt0)
                                ps = pss.tile([128, 2, 512], F32, tag="s")
                                for m_ in range(gw):
                                    jt = jt0 + m_
                                    co = max(0, 128 * (jt - 4 * qb))
                                    nc.tensor.matmul(
                                        ps[:, m_, co:512],
                                        kT[pb : pb + 64, mt, 128 * jt : 128 * (jt + 1)],
                                        qT[pb : pb + 64, mt, 512 * qb + co : 512 * (qb + 1)],
                                        start=True, stop=True,
                                    )
                                if 128 * (jt0 + gw - 1 - 4 * qb) <= 0:
                                    nc.scalar.activation(
                                        u[:, jt0 : jt0 + gw, :], ps[:, 0:gw, :],
                                        AFT.Exp, scale=1.0 / np.sqrt(HD),
                                    )
                                else:
                                    for m_ in range(gw):
                                        jt = jt0 + m_
                                        co = max(0, 128 * (jt - 4 * qb))
                                        nc.scalar.activation(
                                            u[:, jt, co:512], ps[:, m_, co:512],
                                            AFT.Exp, scale=1.0 / np.sqrt(HD),
                                        )
                                        if co > 0:
                                            nc.vector.memset(u[:, jt, 0:co], 0.0)
                                jt0 += gw
                            nc.vector.tensor_tensor(
                                u[:, 4 * qb : 4 * qb + 4, :],
                                u[:, 4 * qb : 4 * qb + 4, :],
                                mask[:], op=AluOpType.mult,
                            )
                            return u

                        def att_pv(hh, qb, u, o_blk):
                            pb = 64 * (hh % 2)
                            mt = hh // 2
                            nkv = 4 * qb + 4
                            po = pso.tile([128, 512], F32, tag="o")
                            for jt in range(nkv):
                                nc.tensor.matmul(
                                    po[0 : HD + 1, :],
                                    v_aug[:, jt, hh, :],
                                    u[:, jt, :],
                                    start=(jt == 0), stop=(jt == nkv - 1),
                                )
                            rz = stg.tile([1, 512], BF16, tag="rz")
                            with nc.allow_low_precision(reason="softmax 1/Z in bf16"):
                                nc.vector.reciprocal(rz[:], po[HD : HD + 1, :])
                            rz_rep = stg.tile([64, 512], BF16, tag="rzr")
                            nc.gpsimd.partition_broadcast(rz_rep[:], rz[:])
                            nc.vector.tensor_tensor(
                                o_blk[pb : pb + 64, mt, :],
                                po[0:HD, :], rz_rep[:], op=AluOpType.mult,
                            )

                        def proj_block(r, o_blk):
                            for t4 in range(4):
                                tsl = slice(512 * r + 128 * t4, 512 * r + 128 * (t4 + 1))
                                st = stg.tile([128, E], BF16, tag="st")
                                for nh in range(2):
                                    psj = pso.tile([128, 512], F32, tag="o")
                                    for mt in range(2):
                                        nc.tensor.matmul(
                                            psj[:],
                                            o_blk[:, mt, 128 * t4 : 128 * (t4 + 1)],
                                            wp_sb[:, mt, 512 * nh : 512 * (nh + 1)],
                                            start=(mt == 0), stop=(mt == 1),
                                        )
                                    nc.vector.tensor_copy(
                                        st[:, 512 * nh : 512 * (nh + 1)], psj[:]
                                    )
                                nc.sync.dma_start(rs_in[tsl, :], st[:])

                        def attention(r):
                            if upto < 5:
                                return
                            o_blk = up.tile([128, 2, 512], BF16, tag="ob")
                            prev = None
                            for hh in range(HL):
                                u = att_scores(hh, r)
                                if prev is not None:
                                    att_pv(prev[0], r, prev[1], o_blk)
                                prev = (hh, u)
                            att_pv(prev[0], r, prev[1], o_blk)
                            if upto >= 6:
                                proj_block(r, o_blk)
                            # h_pb for one token tile per iteration (off critical path)
                            nc.vector.tensor_tensor(
                                z_sb[:, r, :], z_sb[:, r, :], g1_rep[:], op=AluOpType.mult
                            )
                            nc.vector.tensor_tensor(
                                z_sb[:, r, :], z_sb[:, r, :], bb1_rep[:], op=AluOpType.add
                            )

                        # interleave: fill the unb-g1 DMA latency with g0 chains
                        qkv_rank(0, 0)
                        qkv_rank(1, 0)
                        qkv_rank(2, 0)
                        qkv_rank(0, 1)
                        attention(0)
                        qkv_rank(3, 0)
                        qkv_rank(1, 1)
                        attention(1)
                        qkv_rank(2, 1)
                        attention(2)
                        qkv_rank(3, 1)
                        attention(3)

                    if upto < 6:
                        return
                    # ---- ReduceScatter of partial projections ----
                    if single:
                        nc.sync.dma_start(rs_out[:], rs_in[0:TOWN, :])
                    else:
                        nc.gpsimd.collective_compute(
                            "ReduceScatter", AluOpType.add, replica_groups=RG,
                            ins=[rs_in.opt()], outs=[rs_out.opt()],
                        )

            _phase_attn()

            # ---- x2 = h_pb + attn_out; LN2; transpose z2 ----
            if upto >= 7:
                pop_cm = tc.tile_pool(name="post", bufs=1)
                pop = pop_cm.__enter__()
                x2s = pop.tile([128, NT, E], BF16)
                z2T = pop.tile([128, ET, TOWN], BF16)
                g2_rep = pop.tile([128, E], F32)
                bb2_rep = pop.tile([128, E], F32)
                for tt in range(NT):
                    nc.sync.dma_start(
                        x2s[:, tt, :],
                        rs_out[128 * tt : 128 * (tt + 1), :],
                    )
                nc.sync.dma_start(g2_rep[:], g2_d[0:1, :].to_broadcast([128, E]))
                nc.sync.dma_start(bb2_rep[:], bb2_d[0:1, :].to_broadcast([128, E]))
                with tc.tile_pool(name="pst2", bufs=2, space="PSUM") as pst2:
                    for tt in range(NT):
                        nc.vector.tensor_tensor(
                            z_sb[:, tt, :], z_sb[:, tt, :], x2s[:, tt, :], op=AluOpType.add
                        )
                        fn = ln_core if tt % 2 == 0 else ln_core_dve
                        fn(z_sb[:, tt, :], z2_sb[:, tt, :], x2s[:, tt, :], "l2")
                        for et in range(ET):
                            ps = pst2.tile([128, 128], F32, tag="tr2")
                            nc.tensor.transpose(
                                ps[:], z2_sb[:, tt, 128 * et : 128 * (et + 1)], ident[:]
                            )
                            nc.vector.tensor_copy(
                                z2T[:, et, 128 * tt : 128 * (tt + 1)], ps[:]
                            )

            # ---- FFN ----
            if upto >= 9:
                with tc.tile_pool(name="ffp", bufs=1) as fp:
                    nc.sync.dma_start(b1_sb[:], b1_d.rearrange("(ft p) -> p ft", p=128))
                    aT = fp.tile([128, FT, TOWN], BF16)
                    with tc.tile_pool(name="pf", bufs=3, space="PSUM") as pf:
                        for ft in range(FT if upto >= 10 else 0):
                            w1t = w1p.tile([128, ET, 128], BF16, tag="w1")
                            nc.sync.dma_start(
                                w1t[:],
                                w1_d[128 * ft : 128 * (ft + 1), :].rearrange(
                                    "p (kt m) -> p kt m", kt=ET
                                ),
                            )
                            ps = pf.tile([128, 512], F32, tag="f")
                            for kt in range(ET):
                                nc.tensor.matmul(
                                    ps[:], w1t[:, kt, :], z2T[:, kt, :],
                                    start=(kt == 0), stop=(kt == ET - 1),
                                )
                            nc.scalar.activation(
                                aT[:, ft, :], ps[:], AFT.Relu, bias=b1_sb[:, ft : ft + 1]
                            )
                            if ft == 0:
                                # h2b = z2 * g2 + (beta2 + b2), off critical path
                                for tt in range(NT):
                                    nc.vector.tensor_tensor(
                                        z2_sb[:, tt, :], z2_sb[:, tt, :], g2_rep[:],
                                        op=AluOpType.mult,
                                    )
                                    nc.vector.tensor_tensor(
                                        z2_sb[:, tt, :], z2_sb[:, tt, :], bb2_rep[:],
                                        op=AluOpType.add,
                                    )
                    if upto >= 11:
                        with tc.tile_pool(name="pff", bufs=8, space="PSUM") as pff:
                            accs = [
                                pff.tile([128, 512], F32, tag="acc", name=f"acc{i}")
                                for i in range(8)
                            ]
                            for ktf in range(FT):
                                w2t = w2p.tile([128, E], BF16, tag="w2")
                                nc.sync.dma_start(
                                    w2t[:], w2_d[128 * ktf : 128 * (ktf + 1), :]
                                )
                                for tt in range(NT):
                                    for nh in range(2):
                                        nc.tensor.matmul(
                                            accs[2 * tt + nh][:],
                                            aT[:, ktf, 128 * tt : 128 * (tt + 1)],
                                            w2t[:, 512 * nh : 512 * (nh + 1)],
                                            start=(ktf == 0), stop=(ktf == FT - 1),
                                        )
                            for tt in range(NT):
                                for nh in range(2):
                                    sl = slice(512 * nh, 512 * (nh + 1))
                                    nc.vector.tensor_tensor(
                                        z_sb[:, tt, sl], accs[2 * tt + nh][:],
                                        z2_sb[:, tt, sl], op=AluOpType.add,
                                    )
                                nc.sync.dma_start(
                                    out_d[128 * tt : 128 * (tt + 1), :], z_sb[:, tt, :]
                                )
            if upto >= 7:
                pop_cm.__exit__(None, None, None)

    nc.compile()
    return nc


def _in_maps(inputs):
    bf16 = ml_dtypes.bfloat16
    x = np.asarray(inputs["x"], np.float32)
    wq = np.asarray(inputs["wq"], np.float32)
    wk = np.asarray(inputs["wk"], np.float32)
    wv = np.asarray(inputs["wv"], np.float32)
    w_proj = np.asarray(inputs["w_proj"], np.float32)
    b_proj = np.asarray(inputs["b_proj"], np.float32)
    g1 = np.asarray(inputs["gamma1"], np.float32)
    be1 = np.asarray(inputs["beta1"], np.float32)
    g2 = np.asarray(inputs["gamma2"], np.float32)
    be2 = np.asarray(inputs["beta2"], np.float32)
    w1 = np.asarray(inputs["w1"], np.float32)
    b1 = np.asarray(inputs["b1"], np.float32)
    w2 = np.asarray(inputs["w2"], np.float32)
    b2 = np.asarray(inputs["b2"], np.float32)

    wq_f = g1[None, :, None] * wq   # [H, E, hd]
    wk_f = g1[None, :, None] * wk
    wv_f = g1[None, :, None] * wv
    w1_f = g2[:, None] * w1         # [E, dff]
    qb_full = np.einsum("e,hek->hk", be1, wq)   # [H, hd]
    kb_full = np.einsum("e,hek->hk", be1, wk)
    vb_full = np.einsum("e,hek->hk", be1, wv)
    b1a = b1 + be2 @ w1
    be1bp = (be1 + b_proj).reshape(1, E)
    be2b2 = (be2 + b2).reshape(1, E)

    w1r = np.ascontiguousarray(
        w1_f.reshape(ET, 128, FT, 128).transpose(2, 1, 0, 3).reshape(DFF, E)
    ).astype(bf16)
    w2c = np.ascontiguousarray(w2).astype(bf16)
    ident = np.eye(128, dtype=np.float32)
    md = np.zeros((128, 4, 512), np.float32)
    for m in range(4):
        for p_ in range(128):
            cstart = 128 * m + p_
            if cstart < 512:
                md[p_, m, cstart:] = 1.0
    mask_diag = md.reshape(128, 2048).astype(bf16)

    pidx = np.arange(128)
    maps = []
    for c in range(NCORES):
        b, j = c // TP, c % TP
        heads = slice(HL * j, HL * (j + 1))
        qb_s = np.stack(
            [qb_full[heads][2 * mt + pidx // 64, pidx % 64] for mt in range(2)], axis=1
        ).astype(np.float32)
        kb_s = np.stack(
            [kb_full[heads][2 * mt + pidx // 64, pidx % 64] for mt in range(2)], axis=1
        ).astype(np.float32)
        vb_s = np.broadcast_to(
            vb_full[heads].reshape(1, HL * HD), (128, HL * HD)
        ).astype(np.float32)
        wp_rows = np.stack(
            [w_proj[64 * (HL * j + 2 * mt + pidx // 64) + pidx % 64, :] for mt in range(2)],
            axis=1,
        )  # [128, 2, E]
        wp_s = np.ascontiguousarray(wp_rows.reshape(128, 2 * E)).astype(bf16)

        maps.append({
            "x_own": np.ascontiguousarray(x[b, TOWN * j : TOWN * (j + 1)]),
            "wq_s": np.ascontiguousarray(
                wq_f[heads].transpose(1, 0, 2).reshape(E, HL * HD)).astype(bf16),
            "wk_s": np.ascontiguousarray(
                wk_f[heads].transpose(1, 0, 2).reshape(E, HL * HD)).astype(bf16),
            "wv_s": np.ascontiguousarray(
                wv_f[heads].transpose(1, 0, 2).reshape(E, HL * HD)).astype(bf16),
            "qb_s": qb_s, "kb_s": kb_s, "vb_s": np.ascontiguousarray(vb_s),
            "wp_s": wp_s,
            "w1": w1r, "w2": w2c, "b1a": np.ascontiguousarray(b1a),
            "gamma1": g1.reshape(1, E), "be1bp": be1bp,
            "gamma2": g2.reshape(1, E), "be2b2": be2b2,
            "ident": ident, "mask_diag": mask_diag,
        })
    return maps


def kernel(**inputs) -> np.ndarray:
    if "nc" not in _CACHE:
        _CACHE["nc"] = build()
    nc = _CACHE["nc"]
    res = bass_utils.run_bass_kernel_spmd(
        nc, _in_maps(inputs), core_ids=list(range(NCORES))
    )
    out = np.empty((B, T, E), np.float32)
    for c in range(NCORES):
        b, j = c // TP, c % TP
        out[b, TOWN * j : TOWN * (j + 1)] = res.results[c]["out_own"]
    return out


# revision 39
# speedup vs baseline: 1.3904x; 1.0121x over previous
"""Trainium2 Bass kernel for a causal pre-LN decoder block (B=2, T=2048, E=1024,
H=16, hd=64, dff=4096), SPMD over 8 NeuronCores.

Sharding: batch split across the two 4-core groups (cores 0-3 -> batch 0,
cores 4-7 -> batch 1). Within a group, attention is tensor-parallel over heads
(4 heads per core, full sequence); LN, residuals and the FFN are
sequence-parallel (512 tokens per core). Collectives: an AllGather of z^T
(pre-gamma LN output, 2 pipelined rounds of 256 token columns) and an in-group
ReduceScatter(add) of per-core partial attention-output projections (each core
contracts only its own 256 o^T rows against its w_proj row slice, so the
projection matmul does no cross-batch waste).

LayerNorm runs mostly on the Activation engine (Square pass with accumulate
for var, Identity pass with per-token scale/bias for the normalize); gamma1 /
gamma2 are folded into wq/wk/wv/w1 on the host, beta1/beta2 into host-computed
biases, so the QKV and FFN matmuls consume the un-affine z directly.

QKV is interleaved with attention per query block (rank r's QKV chains, then
attention for query block r across all 4 local heads, then the partial
projection of token block r), which keeps PE busy while the Activation engine
works through the softmax Exp backlog.
"""

import numpy as np
import ml_dtypes

import concourse.bacc as bacc
import concourse.mybir as mybir
import concourse.tile as tile
from concourse import bass_utils
from concourse.alu_op_type import AluOpType
from concourse.mybir import ActivationFunctionType as AFT
from bass_rust import AxisListType

B, T, E, H, HD, DFF = 2, 2048, 1024, 16, 64, 4096
NCORES, TP = 8, 4
TOWN = T // TP        # 512 tokens owned per core
NT = TOWN // 128      # 4 own token tiles
ET = E // 128         # 8 tiles along E
KT = T // 128         # 16 kv tiles over full T
HL = H // TP          # 4 local heads
FT = DFF // 128       # 32 tiles along dff
EPS = 1e-5

F32 = mybir.dt.float32
BF16 = mybir.dt.bfloat16
RG = [[0, 1, 2, 3], [4, 5, 6, 7]]

_CACHE = {}


def build(single=False, upto=99):
    ndev = 1 if single else NCORES
    nc = bacc.Bacc("TRN2", target_bir_lowering=False, debug=False, num_devices=ndev)

    def din(name, shape, dt):
        return nc.dram_tensor(name, shape, dt, kind="ExternalInput").ap()

    x_d = din("x_own", [TOWN, E], F32)
    wq_d = din("wq_s", [E, HL * HD], BF16)   # gamma1-folded
    wk_d = din("wk_s", [E, HL * HD], BF16)
    wv_d = din("wv_s", [E, HL * HD], BF16)
    qb_d = din("qb_s", [128, 2], F32)        # beta1 @ wq, per (partition, mt)
    kb_d = din("kb_s", [128, 2], F32)
    vb_d = din("vb_s", [128, HL * HD], F32)  # beta1 @ wv, replicated over partitions
    wp_d = din("wp_s", [128, 2 * E], BF16)   # my w_proj rows, [p, (mt, e)]
    w1_d = din("w1", [DFF, E], BF16)         # gamma2-folded, host-reordered
    w2_d = din("w2", [DFF, E], BF16)
    b1_d = din("b1a", [DFF], F32)            # b1 + beta2 @ w1
    g1_d = din("gamma1", [1, E], F32)
    bb1_d = din("be1bp", [1, E], F32)        # beta1 + b_proj
    g2_d = din("gamma2", [1, E], F32)
    bb2_d = din("be2b2", [1, E], F32)        # beta2 + b2
    id_d = din("ident", [128, 128], F32)
    mk_d = din("mask_diag", [128, 4 * 512], BF16)
    out_d = nc.dram_tensor("out_own", [TOWN, E], F32, kind="ExternalOutput").ap()

    with tile.TileContext(nc) as tc:
        with (
            tc.tile_pool(name="dram", bufs=1, space="DRAM") as dram,
            tc.tile_pool(name="pp", bufs=1) as pp,
            tc.tile_pool(name="lns", bufs=2) as lns,
            tc.tile_pool(name="stg", bufs=2) as stg,
            tc.tile_pool(name="w1s", bufs=4) as w1p,
            tc.tile_pool(name="w2s", bufs=6) as w2p,
        ):
            ag_in = [dram.tile([E, 256], BF16, name=f"agi{g}") for g in range(2)]
            ag_out = [dram.tile([TP * E, 256], BF16, name=f"ago{g}") for g in range(2)]
            rs_in = [dram.tile([TOWN, E], BF16, name=f"rsi{q}") for q in range(4)]
            rs_out = [dram.tile([128, E], BF16, name=f"rso{q}") for q in range(4)]

            # ---- persistent SBUF ----
            ident = pp.tile([128, 128], F32)
            qb_sb = pp.tile([128, 2], F32)
            kb_sb = pp.tile([128, 2], F32)
            vb_sb = pp.tile([128, HL * HD], F32)
            z_sb = pp.tile([128, NT, E], F32)    # z1 -> h_pb -> x2 -> out staging
            z2_sb = pp.tile([128, NT, E], F32)   # LN1 scratch -> z2 -> h2b
            x2s = pp.tile([128, NT, E], BF16)
            g1_rep = pp.tile([128, E], F32)
            bb1_rep = pp.tile([128, E], F32)
            b1_sb = pp.tile([128, FT], F32)

            def ln_core_dve(xin, z_out, scr, tag):
                """DVE-heavy LN (no gamma/beta): keeps the Act engine free."""
                s = lns.tile([128, 1], F32, tag=tag + "ds")
                nc.vector.reduce_sum(s[:], xin, AxisListType.X)
                s2 = lns.tile([128, 1], F32, tag=tag + "d2")
                nc.vector.tensor_tensor(scr, xin, xin, op=AluOpType.mult)
                nc.vector.reduce_sum(s2[:], scr, AxisListType.X)
                m = lns.tile([128, 1], F32, tag=tag + "dm")
                nc.vector.tensor_scalar(m[:], s[:], 1.0 / E, None, op0=AluOpType.mult)
                t = lns.tile([128, 1], F32, tag=tag + "dt")
                nc.vector.tensor_scalar(
                    t[:], s2[:], 1.0 / E, EPS, op0=AluOpType.mult, op1=AluOpType.add
                )
                mm = lns.tile([128, 1], F32, tag=tag + "dmm")
                nc.vector.tensor_tensor(mm[:], m[:], m[:], op=AluOpType.mult)
                veps = lns.tile([128, 1], F32, tag=tag + "de")
                nc.vector.tensor_tensor(veps[:], t[:], mm[:], op=AluOpType.subtract)
                rv = lns.tile([128, 1], F32, tag=tag + "dr")
                nc.vector.reciprocal(rv[:], veps[:])
                rstd = lns.tile([128, 1], F32, tag=tag + "dsr")
                nc.scalar.activation(rstd[:], rv[:], AFT.Sqrt)
                nmean = lns.tile([128, 1], F32, tag=tag + "dnm")
                nc.vector.tensor_scalar(nmean[:], m[:], -1.0, None, op0=AluOpType.mult)
                nc.vector.tensor_scalar(
                    z_out, xin, nmean[:], rstd[:], op0=AluOpType.add, op1=AluOpType.mult
                )

            def ln_core(xin, z_out, scr, tag):
                """z_out = (xin - mean)/sqrt(var+eps); scr is a [128,E] scratch."""
                s = lns.tile([128, 1], F32, tag=tag + "s")
                nc.vector.reduce_sum(s[:], xin, AxisListType.X)
                s2 = lns.tile([128, 1], F32, tag=tag + "s2")
                nc.scalar.activation(scr, xin, AFT.Square, accum_out=s2[:])
                m = lns.tile([128, 1], F32, tag=tag + "m")
                nc.vector.tensor_scalar(m[:], s[:], 1.0 / E, None, op0=AluOpType.mult)
                t = lns.tile([128, 1], F32, tag=tag + "t")
                nc.vector.tensor_scalar(
                    t[:], s2[:], 1.0 / E, EPS, op0=AluOpType.mult, op1=AluOpType.add
                )
                mm = lns.tile([128, 1], F32, tag=tag + "mm")
                nc.vector.tensor_tensor(mm[:], m[:], m[:], op=AluOpType.mult)
                veps = lns.tile([128, 1], F32, tag=tag + "ve")
                nc.vector.tensor_tensor(veps[:], t[:], mm[:], op=AluOpType.subtract)
                rv = lns.tile([128, 1], F32, tag=tag + "rv")
                nc.vector.reciprocal(rv[:], veps[:])
                rstd = lns.tile([128, 1], F32, tag=tag + "rs")
                nc.scalar.activation(rstd[:], rv[:], AFT.Sqrt)
                nmr = lns.tile([128, 1], F32, tag=tag + "nm")
                nc.vector.tensor_scalar(
                    nmr[:], m[:], rstd[:], -1.0, op0=AluOpType.mult, op1=AluOpType.mult
                )
                nc.scalar.activation(z_out, xin, AFT.Identity, bias=nmr[:], scale=rstd[:])

            def _phase_attn():
                with (
                    tc.tile_pool(name="att", bufs=1) as at,
                    tc.tile_pool(name="up", bufs=2) as up,
                ):
                    zT_full = at.tile([128, ET, TP, TOWN], BF16)
                    qT = at.tile([128, 2, T], BF16)
                    kT = at.tile([128, 2, T], BF16)
                    v_aug = at.tile([128, KT, HL, HD + 1], BF16)
                    wq_sb = at.tile([128, ET, HL * HD], BF16)
                    wk_sb = at.tile([128, ET, HL * HD], BF16)
                    wv_sb = at.tile([128, ET, HL * HD], BF16)
                    wp_sb = at.tile([128, 2, E], BF16)
                    mask = at.tile([128, 4 * 512], BF16)

                    # ---- x loads + small weights ----
                    with tc.tile_pool(name="xp", bufs=4) as xp:
                        xts = []
                        for tt in range(NT):
                            xt = xp.tile([128, E], F32, tag="xt")
                            nc.sync.dma_start(xt[:], x_d[128 * tt : 128 * (tt + 1), :])
                            xts.append(xt)
                        nc.sync.dma_start(ident[:], id_d[:])
                        nc.sync.dma_start(qb_sb[:], qb_d[:])
                        nc.sync.dma_start(kb_sb[:], kb_d[:])
                        nc.sync.dma_start(
                            wq_sb[:], wq_d.rearrange("(kt p) m -> p kt m", p=128)
                        )
                        nc.sync.dma_start(
                            wk_sb[:], wk_d.rearrange("(kt p) m -> p kt m", p=128)
                        )

                        # ---- LN1 ----
                        if upto >= 1:
                            for tt in range(NT):
                                fn = ln_core if tt % 2 == 0 else ln_core_dve
                                fn(xts[tt][:], z_sb[:, tt, :], z2_sb[:, tt, :], "l1")

                        # ---- transpose z + AllGather rounds ----
                        def unb(g, r):
                            nc.sync.dma_start(
                                zT_full[:, :, r, 256 * g : 256 * (g + 1)],
                                ag_out[g][E * r : E * (r + 1), :].rearrange(
                                    "(et p) t -> p et t", p=128
                                ),
                            )

                        if upto >= 2:
                            with (
                                tc.tile_pool(name="pst", bufs=2, space="PSUM") as pst,
                                tc.tile_pool(name="ztp", bufs=1) as ztp,
                            ):
                                for g in range(2):
                                    zTo = ztp.tile([128, ET, 256], BF16, tag="zTo")
                                    for lt in range(2):
                                        tt = 2 * g + lt
                                        for et in range(ET):
                                            ps = pst.tile([128, 128], F32, tag="tr")
                                            nc.tensor.transpose(
                                                ps[:],
                                                z_sb[:, tt, 128 * et : 128 * (et + 1)],
                                                ident[:],
                                            )
                                            nc.vector.tensor_copy(
                                                zTo[:, et, 128 * lt : 128 * (lt + 1)], ps[:]
                                            )
                                    nc.sync.dma_start(
                                        ag_in[g].rearrange("(et p) t -> p et t", p=128),
                                        zTo[:],
                                    )
                                    if single:
                                        for r in range(TP):
                                            nc.sync.dma_start(
                                                ag_out[g][E * r : E * (r + 1), :], ag_in[g][:]
                                            )
                                    else:
                                        nc.gpsimd.collective_compute(
                                            "AllGather", AluOpType.bypass, replica_groups=RG,
                                            ins=[ag_in[g].opt()], outs=[ag_out[g].opt()],
                                        )
                                    # unbounce r0 first, interleave weights by need time
                                    unb(g, 0)
                                    if g == 0:
                                        nc.sync.dma_start(
                                            wv_sb[:],
                                            wv_d.rearrange("(kt p) m -> p kt m", p=128),
                                        )
                                        nc.sync.dma_start(vb_sb[:], vb_d[:])
                                    else:
                                        nc.sync.dma_start(
                                            wp_sb[:],
                                            wp_d.rearrange("p (mt e) -> p mt e", mt=2),
                                        )
                                        nc.sync.dma_start(mask[:], mk_d[:])
                                    for r in range(1, TP):
                                        unb(g, r)

                    nc.sync.dma_start(g1_rep[:], g1_d[0:1, :].to_broadcast([128, E]))
                    nc.sync.dma_start(bb1_rep[:], bb1_d[0:1, :].to_broadcast([128, E]))
                    nc.vector.memset(v_aug[:, :, :, HD], 1.0)

                    if upto < 4:
                        return

                    # ---- merged QKV + attention, per rank/query-block r ----
                    with (
                        tc.tile_pool(name="pq", bufs=2, space="PSUM") as pq,
                        tc.tile_pool(name="pss", bufs=2, space="PSUM") as pss,
                        tc.tile_pool(name="pso", bufs=2, space="PSUM") as pso,
                    ):
                        def qkv_rank(r, g):
                            if True:
                                cs = 512 * r + 256 * g
                                for dstT, w_sb, bias, eng in (
                                    (qT, wq_sb, qb_sb, "dve"),
                                    (kT, wk_sb, kb_sb, "dve"),
                                ):
                                    for mt in range(2):
                                        ps = pq.tile([128, 256], F32, tag="qk")
                                        for kt in range(ET):
                                            nc.tensor.matmul(
                                                ps[:],
                                                w_sb[:, kt, 128 * mt : 128 * (mt + 1)],
                                                zT_full[:, kt, r, 256 * g : 256 * (g + 1)],
                                                start=(kt == 0), stop=(kt == ET - 1),
                                            )
                                        dsl = dstT[:, mt, cs : cs + 256]
                                        veng = nc.vector if eng == "dve" else nc.gpsimd
                                        veng.tensor_scalar(
                                            dsl, ps[:], bias[:, mt : mt + 1], None,
                                            op0=AluOpType.add,
                                        )
                                for lt in range(2):
                                    t16 = 4 * r + 2 * g + lt
                                    ps = pq.tile([128, 256], F32, tag="qk")
                                    for kt in range(ET):
                                        nc.tensor.matmul(
                                            ps[:],
                                            zT_full[
                                                :, kt, r,
                                                256 * g + 128 * lt : 256 * g + 128 * (lt + 1),
                                            ],
                                            wv_sb[:, kt, :],
                                            start=(kt == 0), stop=(kt == ET - 1),
                                        )
                                    nc.vector.tensor_tensor(
                                        v_aug[:, t16, :, 0:HD],
                                        ps[:].rearrange("p (hh d) -> p hh d", d=HD),
                                        vb_sb[:].rearrange("p (hh d) -> p hh d", d=HD),
                                        op=AluOpType.add,
                                    )

                        def att_scores(hh, qb):
                            pb = 64 * (hh % 2)
                            mt = hh // 2
                            nkv = 4 * qb + 4
                            u = up.tile([128, KT, 512], BF16, tag="u")
                            jt0 = 0
                            while jt0 < nkv:
                                gw = min(2, nkv - jt0)
                                ps = pss.tile([128, 2, 512], F32, tag="s")
                                for m_ in range(gw):
                                    jt = jt0 + m_
                                    co = max(0, 128 * (jt - 4 * qb))
                                    nc.tensor.matmul(
                                        ps[:, m_, co:512],
                                        kT[pb : pb + 64, mt, 128 * jt : 128 * (jt + 1)],
                                        qT[pb : pb + 64, mt, 512 * qb + co : 512 * (qb + 1)],
                                        start=True, stop=True,
                                    )
                                if 128 * (jt0 + gw - 1 - 4 * qb) <= 0:
                                    nc.scalar.activation(
                                        u[:, jt0 : jt0 + gw, :], ps[:, 0:gw, :],
                                        AFT.Exp, scale=1.0 / np.sqrt(HD),
                                    )
                                else:
                                    for m_ in range(gw):
                                        jt = jt0 + m_
                                        co = max(0, 128 * (jt - 4 * qb))
                                        nc.scalar.activation(
                                            u[:, jt, co:512], ps[:, m_, co:512],
                                            AFT.Exp, scale=1.0 / np.sqrt(HD),
                                        )
                                        if co > 0:
                                            nc.vector.memset(u[:, jt, 0:co], 0.0)
                                jt0 += gw
                            nc.vector.tensor_tensor(
                                u[:, 4 * qb : 4 * qb + 4, :],
                                u[:, 4 * qb : 4 * qb + 4, :],
                                mask[:], op=AluOpType.mult,
                            )
                            return u

                        def att_pv(hh, qb, u, o_blk):
                            pb = 64 * (hh % 2)
                            mt = hh // 2
                            nkv = 4 * qb + 4
                            po = pso.tile([128, 512], F32, tag="o")
                            for jt in range(nkv):
                                nc.tensor.matmul(
                                    po[0 : HD + 1, :],
                                    v_aug[:, jt, hh, :],
                                    u[:, jt, :],
                                    start=(jt == 0), stop=(jt == nkv - 1),
                                )
                            rz = stg.tile([1, 512], BF16, tag="rz")
                            with nc.allow_low_precision(reason="softmax 1/Z in bf16"):
                                nc.vector.reciprocal(rz[:], po[HD : HD + 1, :])
                            rz_rep = stg.tile([64, 512], BF16, tag="rzr")
                            nc.gpsimd.partition_broadcast(rz_rep[:], rz[:])
                            nc.vector.tensor_tensor(
                                o_blk[pb : pb + 64, mt, :],
                                po[0:HD, :], rz_rep[:], op=AluOpType.mult,
                            )

                        def proj_block(r, o_blk):
                            for t4 in range(4):
                                tsl = slice(128 * r, 128 * (r + 1))
                                st = stg.tile([128, E], BF16, tag="st")
                                for nh in range(2):
                                    psj = pso.tile([128, 512], F32, tag="o")
                                    for mt in range(2):
                                        nc.tensor.matmul(
                                            psj[:],
                                            o_blk[:, mt, 128 * t4 : 128 * (t4 + 1)],
                                            wp_sb[:, mt, 512 * nh : 512 * (nh + 1)],
                                            start=(mt == 0), stop=(mt == 1),
                                        )
                                    if r == 3:
                                        nc.scalar.copy(
                                            st[:, 512 * nh : 512 * (nh + 1)], psj[:]
                                        )
                                    else:
                                        nc.vector.tensor_copy(
                                            st[:, 512 * nh : 512 * (nh + 1)], psj[:]
                                        )
                                nc.sync.dma_start(rs_in[t4][tsl, :], st[:])
                                if r == 3 and upto >= 6:
                                    # last writer: kick the collective + read-back now
                                    if single:
                                        nc.sync.dma_start(rs_out[t4][:], rs_in[t4][0:128, :])
                                    else:
                                        nc.gpsimd.collective_compute(
                                            "ReduceScatter", AluOpType.add,
                                            replica_groups=RG,
                                            ins=[rs_in[t4].opt()], outs=[rs_out[t4].opt()],
                                        )
                                    nc.sync.dma_start(x2s[:, t4, :], rs_out[t4][:])

                        def attention(r):
                            if upto < 5:
                                return
                            o_blk = up.tile([128, 2, 512], BF16, tag="ob")
                            prev = None
                            for hh in range(HL):
                                u = att_scores(hh, r)
                                if prev is not None:
                                    att_pv(prev[0], r, prev[1], o_blk)
                                prev = (hh, u)
                            att_pv(prev[0], r, prev[1], o_blk)
                            if upto >= 6:
                                proj_block(r, o_blk)
                            # h_pb off the critical path: tiles r and (3 if r==2)
                            hts = [r] + ([3] if r == 2 else [])
                            if r < 3:
                                for ht in hts:
                                    nc.vector.tensor_tensor(
                                        z_sb[:, ht, :], z_sb[:, ht, :], g1_rep[:],
                                        op=AluOpType.mult,
                                    )
                                    nc.vector.tensor_tensor(
                                        z_sb[:, ht, :], z_sb[:, ht, :], bb1_rep[:],
                                        op=AluOpType.add,
                                    )

                        # interleave: fill the unb-g1 DMA latency with g0 chains
                        qkv_rank(0, 0)
                        qkv_rank(1, 0)
                        qkv_rank(2, 0)
                        qkv_rank(0, 1)
                        attention(0)
                        qkv_rank(3, 0)
                        qkv_rank(1, 1)
                        attention(1)
                        qkv_rank(2, 1)
                        attention(2)
                        qkv_rank(3, 1)
                        attention(3)

                    if upto < 6:
                        return

            _phase_attn()

            # ---- x2 = h_pb + attn_out; LN2; transpose z2 ----
            if upto >= 7:
                pop_cm = tc.tile_pool(name="post", bufs=1)
                pop = pop_cm.__enter__()
                z2T = pop.tile([128, ET, TOWN], BF16)
                g2_rep = pop.tile([128, E], F32)
                bb2_rep = pop.tile([128, E], F32)
                nc.sync.dma_start(g2_rep[:], g2_d[0:1, :].to_broadcast([128, E]))
                nc.sync.dma_start(bb2_rep[:], bb2_d[0:1, :].to_broadcast([128, E]))
                with tc.tile_pool(name="pst2", bufs=2, space="PSUM") as pst2:
                    for tt in range(NT):
                        nc.vector.tensor_tensor(
                            z_sb[:, tt, :], z_sb[:, tt, :], x2s[:, tt, :], op=AluOpType.add
                        )
                        fn = ln_core if tt % 2 == 0 else ln_core_dve
                        fn(z_sb[:, tt, :], z2_sb[:, tt, :], x2s[:, tt, :], "l2")
                        for et in range(ET):
                            ps = pst2.tile([128, 128], F32, tag="tr2")
                            nc.tensor.transpose(
                                ps[:], z2_sb[:, tt, 128 * et : 128 * (et + 1)], ident[:]
                            )
                            if et % 2 == 0:
                                nc.vector.tensor_copy(
                                    z2T[:, et, 128 * tt : 128 * (tt + 1)], ps[:]
                                )
                            else:
                                nc.scalar.copy(
                                    z2T[:, et, 128 * tt : 128 * (tt + 1)], ps[:]
                                )

            # ---- FFN ----
            if upto >= 9:
                with tc.tile_pool(name="ffp", bufs=1) as fp:
                    nc.sync.dma_start(b1_sb[:], b1_d.rearrange("(ft p) -> p ft", p=128))
                    aT = fp.tile([128, FT, TOWN], BF16)
                    with tc.tile_pool(name="pf", bufs=3, space="PSUM") as pf:
                        for ft in range(FT if upto >= 10 else 0):
                            w1t = w1p.tile([128, ET, 128], BF16, tag="w1")
                            nc.sync.dma_start(
                                w1t[:],
                                w1_d[128 * ft : 128 * (ft + 1), :].rearrange(
                                    "p (kt m) -> p kt m", kt=ET
                                ),
                            )
                            ps = pf.tile([128, 512], F32, tag="f")
                            for kt in range(ET):
                                nc.tensor.matmul(
                                    ps[:], w1t[:, kt, :], z2T[:, kt, :],
                                    start=(kt == 0), stop=(kt == ET - 1),
                                )
                            nc.scalar.activation(
                                aT[:, ft, :], ps[:], AFT.Relu, bias=b1_sb[:, ft : ft + 1]
                            )
                            if ft == 0:
                                # h2b = z2 * g2 + (beta2 + b2), off critical path
                                for tt in range(NT):
                                    nc.vector.tensor_tensor(
                                        z2_sb[:, tt, :], z2_sb[:, tt, :], g2_rep[:],
                                        op=AluOpType.mult,
                                    )
                                    nc.vector.tensor_tensor(
                                        z2_sb[:, tt, :], z2_sb[:, tt, :], bb2_rep[:],
                                        op=AluOpType.add,
                                    )
                    if upto >= 11:
                        with tc.tile_pool(name="pff", bufs=8, space="PSUM") as pff:
                            accs = [
                                pff.tile([128, 512], F32, tag="acc", name=f"acc{i}")
                                for i in range(8)
                            ]
                            for ktf in range(FT - 1):
                                w2t = w2p.tile([128, E], BF16, tag="w2")
                                nc.sync.dma_start(
                                    w2t[:], w2_d[128 * ktf : 128 * (ktf + 1), :]
                                )
                                for tt in range(NT):
                                    for nh in range(2):
                                        nc.tensor.matmul(
                                            accs[2 * tt + nh][:],
                                            aT[:, ktf, 128 * tt : 128 * (tt + 1)],
                                            w2t[:, 512 * nh : 512 * (nh + 1)],
                                            start=(ktf == 0), stop=False,
                                        )
                            # last ktf: evacuate each accumulator as it closes
                            ktf = FT - 1
                            w2t = w2p.tile([128, E], BF16, tag="w2")
                            nc.sync.dma_start(w2t[:], w2_d[128 * ktf : 128 * (ktf + 1), :])
                            for tt in range(NT):
                                for nh in range(2):
                                    sl = slice(512 * nh, 512 * (nh + 1))
                                    nc.tensor.matmul(
                                        accs[2 * tt + nh][:],
                                        aT[:, ktf, 128 * tt : 128 * (tt + 1)],
                                        w2t[:, sl],
                                        start=False, stop=True,
                                    )
                                    nc.vector.tensor_tensor(
                                        z_sb[:, tt, sl], accs[2 * tt + nh][:],
                                        z2_sb[:, tt, sl], op=AluOpType.add,
                                    )
                                nc.sync.dma_start(
                                    out_d[128 * tt : 128 * (tt + 1), :], z_sb[:, tt, :]
                                )
            if upto >= 7:
                pop_cm.__exit__(None, None, None)

    nc.compile()
    return nc


def _in_maps(inputs):
    bf16 = ml_dtypes.bfloat16
    x = np.asarray(inputs["x"], np.float32)
    wq = np.asarray(inputs["wq"], np.float32)
    wk = np.asarray(inputs["wk"], np.float32)
    wv = np.asarray(inputs["wv"], np.float32)
    w_proj = np.asarray(inputs["w_proj"], np.float32)
    b_proj = np.asarray(inputs["b_proj"], np.float32)
    g1 = np.asarray(inputs["gamma1"], np.float32)
    be1 = np.asarray(inputs["beta1"], np.float32)
    g2 = np.asarray(inputs["gamma2"], np.float32)
    be2 = np.asarray(inputs["beta2"], np.float32)
    w1 = np.asarray(inputs["w1"], np.float32)
    b1 = np.asarray(inputs["b1"], np.float32)
    w2 = np.asarray(inputs["w2"], np.float32)
    b2 = np.asarray(inputs["b2"], np.float32)

    wq_f = g1[None, :, None] * wq   # [H, E, hd]
    wk_f = g1[None, :, None] * wk
    wv_f = g1[None, :, None] * wv
    w1_f = g2[:, None] * w1         # [E, dff]
    qb_full = np.einsum("e,hek->hk", be1, wq)   # [H, hd]
    kb_full = np.einsum("e,hek->hk", be1, wk)
    vb_full = np.einsum("e,hek->hk", be1, wv)
    b1a = b1 + be2 @ w1
    be1bp = (be1 + b_proj).reshape(1, E)
    be2b2 = (be2 + b2).reshape(1, E)

    w1r = np.ascontiguousarray(
        w1_f.reshape(ET, 128, FT, 128).transpose(2, 1, 0, 3).reshape(DFF, E)
    ).astype(bf16)
    w2c = np.ascontiguousarray(w2).astype(bf16)
    ident = np.eye(128, dtype=np.float32)
    md = np.zeros((128, 4, 512), np.float32)
    for m in range(4):
        for p_ in range(128):
            cstart = 128 * m + p_
            if cstart < 512:
                md[p_, m, cstart:] = 1.0
    mask_diag = md.reshape(128, 2048).astype(bf16)

    pidx = np.arange(128)
    maps = []
    for c in range(NCORES):
        b, j = c // TP, c % TP
        heads = slice(HL * j, HL * (j + 1))
        qb_s = np.stack(
            [qb_full[heads][2 * mt + pidx // 64, pidx % 64] for mt in range(2)], axis=1
        ).astype(np.float32)
        kb_s = np.stack(
            [kb_full[heads][2 * mt + pidx // 64, pidx % 64] for mt in range(2)], axis=1
        ).astype(np.float32)
        vb_s = np.broadcast_to(
            vb_full[heads].reshape(1, HL * HD), (128, HL * HD)
        ).astype(np.float32)
        wp_rows = np.stack(
            [w_proj[64 * (HL * j + 2 * mt + pidx // 64) + pidx % 64, :] for mt in range(2)],
            axis=1,
        )  # [128, 2, E]
        wp_s = np.ascontiguousarray(wp_rows.reshape(128, 2 * E)).astype(bf16)

        maps.append({
            "x_own": np.ascontiguousarray(x[b, TOWN * j : TOWN * (j + 1)]),
            "wq_s": np.ascontiguousarray(
                wq_f[heads].transpose(1, 0, 2).reshape(E, HL * HD)).astype(bf16),
            "wk_s": np.ascontiguousarray(
                wk_f[heads].transpose(1, 0, 2).reshape(E, HL * HD)).astype(bf16),
            "wv_s": np.ascontiguousarray(
                wv_f[heads].transpose(1, 0, 2).reshape(E, HL * HD)).astype(bf16),
            "qb_s": qb_s, "kb_s": kb_s, "vb_s": np.ascontiguousarray(vb_s),
            "wp_s": wp_s,
            "w1": w1r, "w2": w2c, "b1a": np.ascontiguousarray(b1a),
            "gamma1": g1.reshape(1, E), "be1bp": be1bp,
            "gamma2": g2.reshape(1, E), "be2b2": be2b2,
            "ident": ident, "mask_diag": mask_diag,
        })
    return maps


def kernel(**inputs) -> np.ndarray:
    if "nc" not in _CACHE:
        _CACHE["nc"] = build()
    nc = _CACHE["nc"]
    res = bass_utils.run_bass_kernel_spmd(
        nc, _in_maps(inputs), core_ids=list(range(NCORES))
    )
    out = np.empty((B, T, E), np.float32)
    for c in range(NCORES):
        b, j = c // TP, c % TP
        out[b, TOWN * j : TOWN * (j + 1)] = res.results[c]["out_own"]
    return out
